# revision 1
# baseline (speedup 1.0000x reference)
# Trainium2 Bass kernel for nn_AdaptiveCrossHadamard.
#
# Reference computation (per sample):
#   y   = BN(Conv1x1(x))                                  [256, 64*64]
#   p   = mean_pixels(y); logits = conv1d(p, eca_w, k=5)  [256]
#   idx = top_32(logits) (sorted desc, ties -> lower idx)
#   xs  = y[idx]                                          [32, 4096]
#   z   = BN_s(xs[hi] * xs[hj])  for all i<j pairs        [496, 4096]
#   out = concat([y, z], channel axis)                    [752, 4096]
#
# Strategy (8 NeuronCores, batch-parallel, 2 samples/core, no collectives):
#   - BN folded into matmul weights host-side; ECA conv1d as a banded 256x256
#     matrix host-side (values from eca_w, structure static).
#   - y via fp16 matmuls (fp32 accumulate in PSUM), bias added by ScalarE on
#     the PSUM->SBUF copy.
#   - pooled computed EXACTLY in f32 via linearity: mean_pix(y) =
#     W' @ sum_pix(x)/4096 + b'  (f32 row-sums of x fused into the cast pass,
#     f32 matmul). This keeps the top-k ranking faithful to the f32 reference.
#   - top-k as dense ops: rank[c] = #{b!=c: logits[b] > logits[c]} (+ exact
#     tie-break via a lower-triangular mask), one-hot selection matrix S from
#     rank==iota, selected channels as a matmul with x (weights = W' @ S).
#   - pairwise Hadamard: one-hot pair matrices replicate the 32 selected rows
#     to 496 pair rows on the TensorEngine; VectorE does
#     t = (A * scale_s) * B in one scalar_tensor_tensor op (A read from PSUM);
#     the per-pair shift is added on DVE/ACT (alternating) into the staging
#     tile that feeds the output DMA.
#   - constants packed into 3 blob DMAs on the scalar HWDGE queue so the
#     x loads own the sync queue from t=0.
import os
import sys
import numpy as np

_TRN_REPO = "/opt/trn_rl_repo"
if _TRN_REPO not in sys.path and os.path.isdir(_TRN_REPO):
    sys.path.insert(0, _TRN_REPO)

import concourse.bacc as bacc
import concourse.bass as bass
import concourse.mybir as mybir
import concourse.tile as tile
from concourse.bass_utils import run_bass_kernel_spmd

F32 = mybir.dt.float32
F16 = mybir.dt.float16
AF = mybir.ActivationFunctionType
ALU = mybir.AluOpType

B, C1, H, W = 16, 256, 64, 64
PIX = H * W                      # 4096
CS = 32
CSE = CS * (CS - 1) // 2         # 496
NCORES = 8
SPC = B // NCORES                # samples per core = 2
COUT = C1 + CSE                  # 752
EPS = 1e-5

NT = PIX // 512                  # 8 pixel tiles of 512
MT4 = (CSE + 127) // 128         # 4 pair-row tiles (128,128,128,112)
HPIX = PIX // 2

# f32 const blob column layout: [128, CB32]
_WY32 = 0                        # wyT32s, 2 chunks x 256
_CMAT = 512                      # cmat, 2 x 256
_TRIL = 1024                     # tril, 2 x 256
_OFFD = 1536                     # offd, 2 x 256
_BCOL = 2048                     # bcol, 2 x 1
_SCOL = 2050                     # scale_s, 4 x 1
_SHCOL = 2054                    # shift_s, 4 x 1
_IOTA = 2058                     # iota, 32
CB32 = 2090
# fp16 const blob: [128, CB16]
_WY16 = 0                        # wyT16, 2 x 256
_WFOLD = 512                     # wfold16, 2 x 256
_BC16 = 1024                     # bcol16, 2 x 1
CB16 = 1026


def _build(nc: bass.Bass, dbg: bool = False):
    """Emit the per-core Tile program. SPMD: all 8 cores run this graph."""
    x_d = nc.dram_tensor("x16v", [SPC * C1, PIX], F16, kind="ExternalInput")
    xsum_d = nc.dram_tensor("xsumv", [128, 2 * SPC], F32, kind="ExternalInput")
    out_d = nc.dram_tensor("out", [SPC * COUT, PIX], F32, kind="ExternalOutput")
    cb32_d = nc.dram_tensor("cb32", [128, CB32], F32, kind="ExternalInput")
    cb16_d = nc.dram_tensor("cb16", [128, CB16], F16, kind="ExternalInput")
    cbp_d = nc.dram_tensor("cbp", [CS + 1, 2 * CSE], F16, kind="ExternalInput")
    ones_d = nc.dram_tensor("ones_pix", [1, PIX], F16, kind="ExternalInput")
    if dbg:
        dbg_pooled = nc.dram_tensor("dbg_pooled", [SPC * C1, 1], F32,
                                    kind="ExternalOutput")
        dbg_lrow = nc.dram_tensor("dbg_lrow", [SPC, C1], F32,
                                  kind="ExternalOutput")
        dbg_rank = nc.dram_tensor("dbg_rank", [SPC * C1, 1], F32,
                                  kind="ExternalOutput")
        dbg_st = nc.dram_tensor("dbg_st", [SPC * C1, CS], F16,
                                kind="ExternalOutput")
        dbg_xsel = nc.dram_tensor("dbg_xsel", [SPC * CS, PIX], F16,
                                  kind="ExternalOutput")
        dbg_t = nc.dram_tensor("dbg_t", [SPC * 512, PIX], F32,
                               kind="ExternalOutput")

    from contextlib import ExitStack
    with tile.TileContext(nc) as tc, ExitStack() as ctx:
        cpool = ctx.enter_context(tc.tile_pool(name="consts", bufs=1))
        x16p = ctx.enter_context(tc.tile_pool(name="x16", bufs=4))
        yp = ctx.enter_context(tc.tile_pool(name="ysb", bufs=2))
        zp = ctx.enter_context(tc.tile_pool(name="zout", bufs=3))
        sqp = ctx.enter_context(tc.tile_pool(name="sq16", bufs=4))
        xselp = ctx.enter_context(tc.tile_pool(name="xsel", bufs=2))
        gp = ctx.enter_context(tc.tile_pool(name="gwork", bufs=2))
        smallp = ctx.enter_context(tc.tile_pool(name="small", bufs=4))
        # PSUM: pair-phase A/B in [128,1024] tiles (2 banks each, bufs=3)
        # partitioned from the y/xsel/small matmuls so neither starves the
        # other.
        psMM = ctx.enter_context(tc.tile_pool(name="psMM", bufs=2, space="PSUM"))
        psS = ctx.enter_context(tc.tile_pool(name="psS", bufs=3, space="PSUM"))
        psQ = ctx.enter_context(tc.tile_pool(name="psQ", bufs=3, space="PSUM"))

        dma = nc.sync.dma_start

        # ---- x (fp16, pre-cast on host) loads FIRST on the sync queue
        X16 = [[None] * 2 for _ in range(SPC)]
        for s in range(SPC):
            for kt in range(2):
                xt = x16p.tile([128, PIX], F16, name="xt")
                r0 = s * C1 + kt * 128
                dma(out=xt[:, :HPIX], in_=x_d[r0:r0 + 128, :HPIX])
                dma(out=xt[:, HPIX:], in_=x_d[r0:r0 + 128, HPIX:])
                X16[s][kt] = xt
        # ---- constants on the scalar HWDGE queue (xsum first: it gates
        # the top-k chain) ----
        xsumt = cpool.tile([128, 2 * SPC], F32, tag="xsumt")
        nc.scalar.dma_start(out=xsumt[:], in_=xsum_d[:, :])
        cb32 = cpool.tile([128, CB32], F32, tag="cb32")
        nc.scalar.dma_start(out=cb32[:], in_=cb32_d[:, :])
        cb16 = cpool.tile([128, CB16], F16, tag="cb16")
        nc.scalar.dma_start(out=cb16[:], in_=cb16_d[:, :])
        cbp = cpool.tile([CS + 1, 2 * CSE], F16, tag="cbp")
        nc.scalar.dma_start(out=cbp[:], in_=cbp_d[:, :])

        def c32(col, w):
            return cb32[:, col:col + w]

        wyT32s = [c32(_WY32 + k * 256, 256) for k in range(2)]
        cmat = [c32(_CMAT + k * 256, 256) for k in range(2)]
        tril = [c32(_TRIL + k * 256, 256) for k in range(2)]
        offd = [c32(_OFFD + k * 256, 256) for k in range(2)]
        bcol = [c32(_BCOL + k, 1) for k in range(2)]
        scol = [c32(_SCOL + m, 1) for m in range(MT4)]
        shcol = [c32(_SHCOL + m, 1) for m in range(MT4)]
        iota32 = c32(_IOTA, CS)
        wyT16 = [cb16[:, _WY16 + k * 256: _WY16 + (k + 1) * 256] for k in range(2)]
        wfold16 = [cb16[:, _WFOLD + k * 256: _WFOLD + (k + 1) * 256] for k in range(2)]
        bcol16 = [cb16[:, _BC16 + k: _BC16 + k + 1] for k in range(2)]
        pS16 = cbp[0:CS, 0:CSE]
        pQ16 = cbp[0:CS + 1, CSE:2 * CSE]

        WSEL = [None] * SPC
        SBIAS = [None] * SPC
        XSEL = [None] * SPC
        XSQ = [None] * SPC

        def ph_y(s, mt):
            # y = W'x + b' (fp16 matmul, f32 psum), ACT adds bias.
            # nt pairs share one weight load per K-tile.
            if True:
                y_sb = yp.tile([128, PIX], F32)
                for ntp in range(NT // 2):
                    yps = [psMM.tile([128, 512], F32, tag="mm", name=f"yps{j}")
                           for j in range(2)]
                    for kt in range(2):
                        for j in range(2):
                            nt = ntp * 2 + j
                            nc.tensor.matmul(
                                yps[j][:],
                                lhsT=wyT16[kt][:, mt * 128:(mt + 1) * 128],
                                rhs=X16[s][kt][:, nt * 512:(nt + 1) * 512],
                                start=(kt == 0), stop=(kt == 1))
                    for j in range(2):
                        nt = ntp * 2 + j
                        nc.scalar.activation(
                            y_sb[:, nt * 512:(nt + 1) * 512], yps[j][:],
                            AF.Identity, bias=bcol[mt], scale=1.0)
                dma(out=out_d[s * COUT + mt * 128: s * COUT + (mt + 1) * 128, :],
                    in_=y_sb[:])

        def ph_sel(s):
            # pooled = W'@xbar + b' (exact f32; wyT32s folds the /4096)
            pooled = []
            for mt in range(2):
                pp = psMM.tile([128, 1], F32, tag="mm")
                for kt in range(2):
                    nc.tensor.matmul(
                        pp[:], lhsT=wyT32s[kt][:, mt * 128:(mt + 1) * 128],
                        rhs=xsumt[:, s * 2 + kt: s * 2 + kt + 1],
                        start=(kt == 0), stop=(kt == 1))
                pb = smallp.tile([128, 1], F32, tag="pooled")
                nc.scalar.activation(pb[:], pp[:], AF.Identity,
                                     bias=bcol[mt], scale=1.0)
                pooled.append(pb)
                if dbg:
                    dma(out=dbg_pooled[s * C1 + mt * 128:
                                       s * C1 + (mt + 1) * 128, :], in_=pb[:])

            lr_ps = psMM.tile([1, C1], F32, tag="mm")
            for ot in range(2):
                nc.tensor.matmul(lr_ps[:], lhsT=pooled[ot][:], rhs=cmat[ot],
                                 start=(ot == 0), stop=(ot == 1))
            lrow = smallp.tile([1, C1], F32, tag="lrow")
            nc.scalar.copy(lrow[:], lr_ps[:])
            if dbg:
                dma(out=dbg_lrow[s:s + 1, :], in_=lrow[:])

            st = []
            for qt in range(2):
                lc_ps = psMM.tile([128, 1], F32, tag="mm")
                for ot in range(2):
                    nc.tensor.matmul(
                        lc_ps[:], lhsT=cmat[ot][:, qt * 128:(qt + 1) * 128],
                        rhs=pooled[ot][:], start=(ot == 0), stop=(ot == 1))
                lcol = smallp.tile([128, 1], F32, tag="lcol")
                nc.scalar.copy(lcol[:], lc_ps[:])

                # exact broadcast of logits row to all partitions (no PE fp32
                # rounding: the fp32 PE path is ~1e-7 lossy, which made the
                # diagonal compare Brow[a,a] vs lcol[a] misfire)
                brow = gp.tile([128, C1], F32, tag="brow")
                nc.gpsimd.partition_broadcast(brow[:], lrow[:])
                # rank[a] = #{b!=a: logits[b] > logits[a]}
                #        + #{b < a: logits[b] == logits[a]}   (jax tie-break)
                g2 = gp.tile([128, C1], F32)
                nc.vector.scalar_tensor_tensor(
                    g2[:], brow[:], lcol[:], tril[qt],
                    op0=ALU.is_equal, op1=ALU.mult)
                gsum = gp.tile([128, C1], F32)
                nc.vector.scalar_tensor_tensor(
                    gsum[:], brow[:], lcol[:], g2[:],
                    op0=ALU.is_gt, op1=ALU.add)
                gm = gp.tile([128, C1], F32)
                nc.vector.tensor_tensor(gm[:], gsum[:], offd[qt], op=ALU.mult)
                rank = smallp.tile([128, 1], F32, tag="rank")
                nc.vector.tensor_reduce(rank[:], gm[:],
                                        axis=mybir.AxisListType.X, op=ALU.add)
                # S_T[c, k] = (rank[c] == k)
                stq = smallp.tile([128, CS], F16, tag="st")
                nc.vector.tensor_scalar(stq[:], iota32, rank[:], None,
                                        op0=ALU.is_equal)
                st.append(stq)
                if dbg:
                    r0 = s * C1 + qt * 128
                    dma(out=dbg_rank[r0:r0 + 128, :], in_=rank[:])
                    dma(out=dbg_st[r0:r0 + 128, :], in_=stq[:])

            # selection weights: W_selT[c,k] = sum_o W'[o,c] S_T[o,k]
            wsel = []
            for ct in range(2):
                ws_ps = psMM.tile([128, CS], F32, tag="mm")
                for ot in range(2):
                    nc.tensor.matmul(
                        ws_ps[:], lhsT=wfold16[ot][:, ct * 128:(ct + 1) * 128],
                        rhs=st[ot][:], start=(ot == 0), stop=(ot == 1))
                wsq = smallp.tile([128, CS], F16, tag="wsel")
                nc.scalar.copy(wsq[:], ws_ps[:])
                wsel.append(wsq)
            WSEL[s] = wsel
            sb_ps = psMM.tile([CS, 1], F32, tag="mm")
            for ot in range(2):
                nc.tensor.matmul(sb_ps[:], lhsT=st[ot][:], rhs=bcol16[ot],
                                 start=(ot == 0), stop=(ot == 1))
            sbias = smallp.tile([CS, 1], F32, tag="sbias")
            nc.scalar.copy(sbias[:], sb_ps[:])
            SBIAS[s] = sbias

        def ph_xsel(s):
            # x_sel = W_sel @ x + S b'  (fp16, straight from x)
            xsel = xselp.tile([CS, PIX], F16)
            for ntp in range(NT // 2):
                xps = [psMM.tile([CS, 512], F32, tag="mm", name=f"xps{j}") for j in range(2)]
                for kt in range(2):
                    for j in range(2):
                        nt = ntp * 2 + j
                        nc.tensor.matmul(
                            xps[j][:], lhsT=WSEL[s][kt][:],
                            rhs=X16[s][kt][:, nt * 512:(nt + 1) * 512],
                            start=(kt == 0), stop=(kt == 1))
                for j in range(2):
                    nt = ntp * 2 + j
                    nc.scalar.activation(xsel[:, nt * 512:(nt + 1) * 512],
                                         xps[j][:], AF.Identity,
                                         bias=SBIAS[s][:], scale=1.0)
            XSEL[s] = xsel
            # xsq_aug: rows 0-31 = xsel^2 (fp16), row 32 = 1.0 (carries the
            # -shift row of pQ through the Q matmul)
            xsq = xselp.tile([CS + 1, PIX], F16, tag="xsq", name="xsq")
            dma(out=xsq[CS:CS + 1, :], in_=ones_d[0:1, :])
            for h in range(2):
                nc.scalar.activation(xsq[0:CS, h * HPIX:(h + 1) * HPIX],
                                     xsel[:, h * HPIX:(h + 1) * HPIX],
                                     AF.Square)
            XSQ[s] = xsq
            if dbg:
                dma(out=dbg_xsel[s * CS:(s + 1) * CS, :], in_=xsel[:])

        def ph_z(s, m, half):
            # z = Square(sqrt(s/2)(xi+xj) @ pS) - [ (s/2)(xi^2+xj^2) - shift ]
            # PE: S and Q matmuls; ACT: Square (doubles as PSUM->SBUF move);
            # DVE: one subtract into the output staging tile.
            p = min(128, CSE - m * 128)
            xsel = XSEL[s]
            xsq = XSQ[s]
            zo = zp.tile([128, HPIX], F32)
            for ntp in range(2):
                sps = [psS.tile([128, 512], F32, tag="sps", name=f"sps{j}")
                       for j in range(2)]
                qps = [psQ.tile([128, 512], F32, tag="qps", name=f"qps{j}")
                       for j in range(2)]
                for j in range(2):
                    nt = half * 4 + ntp * 2 + j
                    nc.tensor.matmul(sps[j][:p, :],
                                     lhsT=pS16[:, m * 128: m * 128 + p],
                                     rhs=xsel[:, nt * 512:(nt + 1) * 512],
                                     start=True, stop=True)
                for j in range(2):
                    nt = half * 4 + ntp * 2 + j
                    nc.tensor.matmul(qps[j][:p, :],
                                     lhsT=pQ16[:, m * 128: m * 128 + p],
                                     rhs=xsq[:, nt * 512:(nt + 1) * 512],
                                     start=True, stop=True)
                for j in range(2):
                    nt2 = ntp * 2 + j
                    sq = sqp.tile([128, 512], F16, tag="sq", name="sq")
                    nc.scalar.activation(sq[:p, :], sps[j][:p, :], AF.Square)
                    # z = (Q * -1) + sq  — same operand pattern as the
                    # hardware-proven stt (PSUM f32 in0, fp16 SBUF in1)
                    nc.vector.scalar_tensor_tensor(
                        zo[:p, nt2 * 512:(nt2 + 1) * 512],
                        qps[j][:p, :], -1.0, sq[:p, :],
                        op0=ALU.mult, op1=ALU.add)
            if dbg:
                dma(out=dbg_t[s * 512 + m * 128: s * 512 + m * 128 + p,
                              half * HPIX:(half + 1) * HPIX],
                    in_=zo[:p, :])
            r0 = s * COUT + C1 + m * 128
            dma(out=out_d[r0:r0 + p, half * HPIX:(half + 1) * HPIX],
                in_=zo[:p, :])

        # emission order == per-engine FIFO order; this ordering measured
        # best: selection chains early, y fills, z units interleaved
        ph_sel(0)
        ph_xsel(0)
        ph_sel(1)
        ph_z(0, 0, 0)
        ph_z(0, 0, 1)
        ph_y(0, 0)
        ph_y(0, 1)
        ph_xsel(1)
        ph_z(0, 1, 0)
        ph_z(0, 1, 1)
        ph_z(1, 0, 0)
        ph_z(1, 0, 1)
        ph_y(1, 0)
        ph_y(1, 1)
        ph_z(1, 1, 0)
        ph_z(1, 1, 1)
        for m, half in [(2, 0), (2, 1), (3, 0), (3, 1)]:
            ph_z(0, m, half)
            ph_z(1, m, half)

_CACHE = {}


def _get_nc(dbg: bool = False):
    key = f"nc{int(dbg)}"
    if key not in _CACHE:
        nc = bacc.Bacc("TRN2", target_bir_lowering=False, debug=False,
                       num_devices=NCORES)
        _build(nc, dbg=dbg)
        nc.compile()
        _CACHE[key] = nc
    return _CACHE[key]


def _host_params(w_fc, b_fc, g_x, b_x, m_x, v_x, eca_w, g_s, b_s, m_s, v_s):
    sx = (g_x / np.sqrt(v_x + EPS)).astype(np.float32)            # [256]
    Wp = (sx[:, None] * w_fc).astype(np.float32)                  # [o, c]
    bp = (sx * b_fc + b_x - m_x * sx).astype(np.float32)          # [256]

    cmat = np.zeros((C1, C1), np.float32)                         # [o, q]
    for k in range(5):
        d = k - 2                                                 # o - q
        for q in range(C1):
            o = q + d
            if 0 <= o < C1:
                cmat[o, q] = eca_w[k]

    tril = (np.arange(C1)[None, :] < np.arange(C1)[:, None]).astype(np.float32)

    hi, hj = np.triu_indices(CS, k=1)
    ss = (g_s / np.sqrt(v_s + EPS)).astype(np.float32)
    sh = (b_s - m_s * ss).astype(np.float32)
    # squares-trick pair matrices:
    #   S' = pS.T @ xsel with pS[i,pq] = sqrt(s/2) * [i in (hi,hj)]
    #   Q  = pQ.T @ [xsel^2; 1] with pQ[i,pq] = (s/2)*[i in (hi,hj)],
    #        pQ[32,pq] = -shift  =>  z = S'^2 - Q
    ar = np.arange(CSE)
    pS = np.zeros((CS, CSE), np.float32)
    pS[hi, ar] = 1.0
    pS[hj, ar] += 1.0
    pS16 = (pS * np.sqrt(ss / 2.0)[None, :]).astype(np.float16)
    pQ = np.zeros((CS + 1, CSE), np.float32)
    pQ[hi, ar] = 1.0
    pQ[hj, ar] += 1.0
    pQ[:CS] *= (ss / 2.0)[None, :]
    pQ[CS] = -sh
    pQ16 = pQ.astype(np.float16)

    return {
        "wyT16": Wp.T.astype(np.float16).copy(),
        "wyT32s": (Wp.T / PIX).astype(np.float32).copy(),
        "wfold16": Wp.astype(np.float16).copy(),
        "bcol": bp.reshape(C1, 1).copy(),
        "bcol16": bp.astype(np.float16).reshape(C1, 1).copy(),
        "cmat": cmat,
        "tril": tril,
        "offd": (1.0 - np.eye(C1, dtype=np.float32)),
        "iota32": np.tile(np.arange(CS, dtype=np.float32), (128, 1)).copy(),
        "pS16": pS16,
        "pQ16": pQ16,
        "scol": ss.reshape(CSE, 1).copy(),
        "shcol": sh.reshape(CSE, 1).copy(),
    }


def _semantic_params(inputs):
    return _host_params(
        np.asarray(inputs["w_fc"], np.float32),
        np.asarray(inputs["b_fc"], np.float32),
        np.asarray(inputs["bn_x_gamma"], np.float32),
        np.asarray(inputs["bn_x_beta"], np.float32),
        np.asarray(inputs["bn_x_mean"], np.float32),
        np.asarray(inputs["bn_x_var"], np.float32),
        np.asarray(inputs["eca_w"], np.float32),
        np.asarray(inputs["bn_s_gamma"], np.float32),
        np.asarray(inputs["bn_s_beta"], np.float32),
        np.asarray(inputs["bn_s_mean"], np.float32),
        np.asarray(inputs["bn_s_var"], np.float32),
    )


def _pack_blobs(P):
    """Pack semantic params into the const blobs matching _build's layout."""
    cb32 = np.zeros((128, CB32), np.float32)
    for k in range(2):
        cb32[:, _WY32 + k * 256: _WY32 + (k + 1) * 256] = \
            P["wyT32s"][k * 128:(k + 1) * 128]
        cb32[:, _CMAT + k * 256: _CMAT + (k + 1) * 256] = \
            P["cmat"][k * 128:(k + 1) * 128]
        cb32[:, _TRIL + k * 256: _TRIL + (k + 1) * 256] = \
            P["tril"][k * 128:(k + 1) * 128]
        cb32[:, _OFFD + k * 256: _OFFD + (k + 1) * 256] = \
            P["offd"][k * 128:(k + 1) * 128]
        cb32[:, _BCOL + k] = P["bcol"][k * 128:(k + 1) * 128, 0]
    for m in range(MT4):
        p = min(128, CSE - m * 128)
        cb32[:p, _SCOL + m] = P["scol"][m * 128: m * 128 + p, 0]
        cb32[:p, _SHCOL + m] = P["shcol"][m * 128: m * 128 + p, 0]
    cb32[:, _IOTA:_IOTA + CS] = P["iota32"]

    cb16 = np.zeros((128, CB16), np.float16)
    for k in range(2):
        cb16[:, _WY16 + k * 256: _WY16 + (k + 1) * 256] = \
            P["wyT16"][k * 128:(k + 1) * 128]
        cb16[:, _WFOLD + k * 256: _WFOLD + (k + 1) * 256] = \
            P["wfold16"][k * 128:(k + 1) * 128]
        cb16[:, _BC16 + k] = P["bcol16"][k * 128:(k + 1) * 128, 0]

    cbp = np.zeros((CS + 1, 2 * CSE), np.float16)
    cbp[0:CS, 0:CSE] = P["pS16"]
    cbp[0:CS + 1, CSE:2 * CSE] = P["pQ16"]
    return {"cb32": cb32, "cb16": np.ascontiguousarray(cb16),
            "cbp": np.ascontiguousarray(cbp),
            "ones_pix": np.ones((1, PIX), np.float16)}


def _in_maps(inputs):
    x = np.ascontiguousarray(np.asarray(inputs["x"], np.float32))
    blobs = _pack_blobs(_semantic_params(inputs))
    maps = []
    for c in range(NCORES):
        shard = x[c * SPC:(c + 1) * SPC].reshape(SPC * C1, PIX)
        # exact f32 per-channel pixel sums (feeds the pooled/top-k path);
        # fp16 x feeds the matmuls
        xsum = shard.sum(axis=1, dtype=np.float32)          # [512]
        xsumv = np.zeros((128, 2 * SPC), np.float32)
        for s in range(SPC):
            for kt in range(2):
                xsumv[:, s * 2 + kt] = xsum[s * C1 + kt * 128:
                                            s * C1 + (kt + 1) * 128]
        maps.append({"x16v": shard.astype(np.float16),
                     "xsumv": xsumv, **blobs})
    return maps


def _ensure_ntff_hook():
    """The agent image lacks antenv.axon_hooks; synthesize it so
    run_bass_kernel_spmd(trace=True) can reach the NTFF profiler in
    libaxon_pjrt.so. Safe no-op if anything is missing."""
    try:
        import antenv.axon_hooks  # noqa: F401
        return
    except ImportError:
        pass
    try:
        import types
        import antenv
        from trn_agent_boot.trn_boot import _ntff_profile_via_ctypes
        hook = _ntff_profile_via_ctypes("/opt/axon/libaxon_pjrt.so")
        mod = types.ModuleType("antenv.axon_hooks")
        mod._hook = hook
        mod.get_axon_ntff_profile_hook = lambda: mod._hook
        mod.set_axon_ntff_profile_hook = lambda h: setattr(mod, "_hook", h)
        sys.modules["antenv.axon_hooks"] = mod
        antenv.axon_hooks = mod
    except Exception as e:  # pragma: no cover
        print(f"ntff hook shim failed: {e}", file=sys.stderr)


def run(inputs, trace=False, dbg=False):
    if trace:
        _ensure_ntff_hook()
    nc = _get_nc(dbg=dbg)
    maps = _in_maps(inputs)
    res = run_bass_kernel_spmd(nc, maps, core_ids=list(range(NCORES)),
                               trace=trace)
    outs = [np.asarray(res.results[c]["out"], np.float32)
            .reshape(SPC, COUT, H, W) for c in range(NCORES)]
    return np.concatenate(outs, axis=0), res


def kernel(**inputs) -> np.ndarray:
    out, _ = run(inputs, trace=False)
    return out



# revision 12
# speedup vs baseline: 1.0211x; 1.0211x over previous
# Trainium2 Bass kernel for nn_AdaptiveCrossHadamard (v2).
#
# Reference computation (per sample):
#   y   = BN(Conv1x1(x))                                  [256, 64*64]
#   p   = mean_pixels(y); logits = conv1d(p, eca_w, k=5)  [256]
#   idx = top_32(logits) (sorted desc, ties -> lower idx)
#   xs  = y[idx]                                          [32, 4096]
#   z   = BN_s(xs[hi] * xs[hj])  for all i<j pairs        [496, 4096]
#   out = concat([y, z], channel axis)                    [752, 4096]
#
# v2 strategy (8 NeuronCores, batch-parallel, 2 samples/core):
#   - fp16 output staging + DMA (halves HBM write traffic); host casts to f32.
#   - xsel computed 4x-replicated across partition groups (Wsel4 weights make
#     the PE produce 4 copies for free); xsq4 = xsel4^2 on DVE/GPSIMD.
#   - pair phase: per 128-row tile m a composite [128,128] weight holds the
#     S one-hot (sum) block twice (rows 0-63) and the negated Q one-hot
#     (square-sum) block twice (rows 64-127).  Four row-tiled matmuls
#     (tile_position) run concurrently: two S windows + two Q windows per
#     span, writing [128,1024] S tiles and a [128,2048] Q tile.
#   - z = (Qneg + sh) + S^2: ACT does Square(S)->fp16, DVE does one
#     scalar_tensor_tensor per [128,2048].
#   - y bias-move split between ACT and DVE (tunable) to balance engines.
#   - input DMAs ride the scalar HWDGE queue; outputs own the sync queue.
import os
import sys
import numpy as np

_TRN_REPO = "/opt/trn_rl_repo"
if _TRN_REPO not in sys.path and os.path.isdir(_TRN_REPO):
    sys.path.insert(0, _TRN_REPO)

import concourse.bacc as bacc
import concourse.bass as bass
import concourse.mybir as mybir
import concourse.tile as tile
from concourse.bass_utils import run_bass_kernel_spmd

F32 = mybir.dt.float32
F16 = mybir.dt.float16
AF = mybir.ActivationFunctionType
ALU = mybir.AluOpType

B, C1, H, W = 16, 256, 64, 64
PIX = H * W                      # 4096
CS = 32
CSE = CS * (CS - 1) // 2         # 496
NCORES = 8
SPC = B // NCORES                # samples per core = 2
COUT = C1 + CSE                  # 752
EPS = 1e-5
MT4 = (CSE + 127) // 128         # 4 pair-row tiles (128,128,128,112)

# f32 const blob column layout: [128, CB32]
_WY32 = 0                        # wyT32s, 2 chunks x 256
_CMAT = 512                      # cmat, 2 x 256
_TRIL = 1024                     # tril, 2 x 256
_OFFD = 1536                     # offd (1 - eye), 2 x 256
_BCOL = 2048                     # bcol, 2 x 1
_SHCOL = 2050                    # shift_s per m-tile, 4 x 1
_IOTA = 2054                     # iota128 (j % 32), 128
CB32 = 2182
# fp16 const blob: [128, CB16]
_WY16 = 0                        # wyT16, 2 x 256
_WFOLD = 512                     # wfold16, 2 x 256
_BC16 = 1024                     # bcol16, 2 x 1
_WPAIR = 1026                    # pair composites, 4 x 128
CB16 = 2050


def _build(nc: bass.Bass, dbg: bool = False):
    """Emit the per-core Tile program. SPMD: all 8 cores run this graph."""
    x_d = nc.dram_tensor("x16v", [SPC * C1, PIX], F16, kind="ExternalInput")
    xsum_d = nc.dram_tensor("xsumv", [128, 2 * SPC], F32, kind="ExternalInput")
    out_d = nc.dram_tensor("out", [SPC * COUT, PIX], F16, kind="ExternalOutput")
    cb32_d = nc.dram_tensor("cb32", [128, CB32], F32, kind="ExternalInput")
    cb16_d = nc.dram_tensor("cb16", [128, CB16], F16, kind="ExternalInput")
    if dbg:
        dbg_rank = nc.dram_tensor("dbg_rank", [SPC * C1, 1], F32,
                                  kind="ExternalOutput")
        dbg_xsel4 = nc.dram_tensor("dbg_xsel4", [SPC * 128, PIX], F16,
                                   kind="ExternalOutput")

    from contextlib import ExitStack
    with tile.TileContext(nc) as tc, ExitStack() as ctx:
        cpool = ctx.enter_context(tc.tile_pool(name="consts", bufs=1))
        x16p = ctx.enter_context(tc.tile_pool(name="x16", bufs=4))
        xselp = ctx.enter_context(tc.tile_pool(name="xsel4", bufs=2))
        xsqp = ctx.enter_context(tc.tile_pool(name="xsq4", bufs=2))
        sqp = ctx.enter_context(tc.tile_pool(name="sq16", bufs=2))
        zp = ctx.enter_context(tc.tile_pool(name="zout", bufs=3))
        ybp = ctx.enter_context(tc.tile_pool(name="ysb", bufs=2))
        gp = ctx.enter_context(tc.tile_pool(name="gwork", bufs=2))
        smallp = ctx.enter_context(tc.tile_pool(name="small", bufs=4))
        # PSUM: tag "s" = [128,1024] f32 (2 banks) x2 bufs; tag "q" =
        # [128,2048] f32 (4 banks) x1 buf.  8 banks total.
        ps = ctx.enter_context(tc.tile_pool(name="ps", bufs=2, space="PSUM"))

        dma_out = nc.sync.dma_start          # outputs own the sync queue
        dma_in = nc.scalar.dma_start         # inputs on the ACT HWDGE queue

        # ---- constants first on the input queue (they gate the top-k
        # chain), then x tiles ----
        xsumt = cpool.tile([128, 2 * SPC], F32, tag="xsumt")
        dma_in(out=xsumt[:], in_=xsum_d[:, :])
        cb32 = cpool.tile([128, CB32], F32, tag="cb32")
        dma_in(out=cb32[:], in_=cb32_d[:, :])
        cb16 = cpool.tile([128, CB16], F16, tag="cb16")
        dma_in(out=cb16[:], in_=cb16_d[:, :])

        HPIX = PIX // 2
        X16 = [[None] * 2 for _ in range(SPC)]
        for s in range(SPC):
            for kt in range(2):
                xt = x16p.tile([128, PIX], F16, name="xt")
                r0 = s * C1 + kt * 128
                dma_in(out=xt[:, :HPIX], in_=x_d[r0:r0 + 128, :HPIX])
                dma_in(out=xt[:, HPIX:], in_=x_d[r0:r0 + 128, HPIX:])
                X16[s][kt] = xt

        def c32(col, w):
            return cb32[:, col:col + w]

        wyT32s = [c32(_WY32 + k * 256, 256) for k in range(2)]
        cmat = [c32(_CMAT + k * 256, 256) for k in range(2)]
        tril = [c32(_TRIL + k * 256, 256) for k in range(2)]
        offd = [c32(_OFFD + k * 256, 256) for k in range(2)]
        bcol = [c32(_BCOL + k, 1) for k in range(2)]
        shcol = [c32(_SHCOL + m, 1) for m in range(MT4)]
        iota128 = c32(_IOTA, 128)
        wyT16 = [cb16[:, _WY16 + k * 256: _WY16 + (k + 1) * 256]
                 for k in range(2)]
        wfold16 = [cb16[:, _WFOLD + k * 256: _WFOLD + (k + 1) * 256]
                   for k in range(2)]
        bcol16 = [cb16[:, _BC16 + k: _BC16 + k + 1] for k in range(2)]
        wpair = [cb16[:, _WPAIR + m * 128: _WPAIR + (m + 1) * 128]
                 for m in range(MT4)]

        WSEL4 = [None] * SPC
        SBIAS4 = [None] * SPC
        XSEL4 = [None] * SPC
        XSQ4 = [None] * SPC
        YSB = [[None] * 2 for _ in range(SPC)]
        ZO = [[None] * MT4 for _ in range(SPC)]

        def ph_sel(s):
            # pooled = W'@xbar + b' (exact f32; wyT32s folds the /4096)
            pooled = []
            for mt in range(2):
                pp = ps.tile([128, 1], F32, tag="s")
                for kt in range(2):
                    nc.tensor.matmul(
                        pp[:], lhsT=wyT32s[kt][:, mt * 128:(mt + 1) * 128],
                        rhs=xsumt[:, s * 2 + kt: s * 2 + kt + 1],
                        start=(kt == 0), stop=(kt == 1))
                pb = smallp.tile([128, 1], F32, tag="pooled")
                nc.scalar.activation(pb[:], pp[:], AF.Identity,
                                     bias=bcol[mt], scale=1.0)
                pooled.append(pb)

            lr_ps = ps.tile([1, C1], F32, tag="s")
            for ot in range(2):
                nc.tensor.matmul(lr_ps[:], lhsT=pooled[ot][:], rhs=cmat[ot],
                                 start=(ot == 0), stop=(ot == 1))
            lrow = smallp.tile([1, C1], F32, tag="lrow")
            nc.scalar.copy(lrow[:], lr_ps[:])

            st4 = []
            for qt in range(2):
                lc_ps = ps.tile([128, 1], F32, tag="s")
                for ot in range(2):
                    nc.tensor.matmul(
                        lc_ps[:], lhsT=cmat[ot][:, qt * 128:(qt + 1) * 128],
                        rhs=pooled[ot][:], start=(ot == 0), stop=(ot == 1))
                lcol = smallp.tile([128, 1], F32, tag="lcol")
                nc.scalar.copy(lcol[:], lc_ps[:])

                # exact broadcast of logits row to all partitions
                brow = gp.tile([128, C1], F32, tag="brow")
                nc.gpsimd.partition_broadcast(brow[:], lrow[:])
                # rank[a] = #{b!=a: logits[b] > logits[a]}
                #        + #{b < a: logits[b] == logits[a]}   (jax tie-break)
                # offd kills the diagonal: lrow/lcol come from different
                # matmuls so brow[a,a] vs lcol[a] can misfire by 1 ulp.
                g2 = gp.tile([128, C1], F32)
                nc.vector.scalar_tensor_tensor(
                    g2[:], brow[:], lcol[:], tril[qt],
                    op0=ALU.is_equal, op1=ALU.mult)
                gsum = gp.tile([128, C1], F32)
                nc.vector.scalar_tensor_tensor(
                    gsum[:], brow[:], lcol[:], g2[:],
                    op0=ALU.is_gt, op1=ALU.add)
                gm = gp.tile([128, C1], F32)
                nc.vector.tensor_tensor(gm[:], gsum[:], offd[qt], op=ALU.mult)
                rank = smallp.tile([128, 1], F32, tag="rank")
                nc.vector.tensor_reduce(rank[:], gm[:],
                                        axis=mybir.AxisListType.X, op=ALU.add)
                # S4_T[c, 32g+k] = (rank[c] == k), 4 horizontal copies
                stq = smallp.tile([128, 128], F16, tag="st", bufs=4)
                nc.vector.tensor_scalar(stq[:], iota128, rank[:], None,
                                        op0=ALU.is_equal)
                st4.append(stq)
                if dbg:
                    r0 = s * C1 + qt * 128
                    dma_out(out=dbg_rank[r0:r0 + 128, :], in_=rank[:])

            # selection weights (4 copies): W_sel4T[c, 32g+k]
            wsel4 = []
            for ct in range(2):
                ws_ps = ps.tile([128, 128], F32, tag="s")
                for ot in range(2):
                    nc.tensor.matmul(
                        ws_ps[:], lhsT=wfold16[ot][:, ct * 128:(ct + 1) * 128],
                        rhs=st4[ot][:], start=(ot == 0), stop=(ot == 1))
                wsq = smallp.tile([128, 128], F16, tag="wsel", bufs=4)
                nc.scalar.copy(wsq[:], ws_ps[:])
                wsel4.append(wsq)
            WSEL4[s] = wsel4
            sb_ps = ps.tile([128, 1], F32, tag="s")
            for ot in range(2):
                nc.tensor.matmul(sb_ps[:], lhsT=st4[ot][:], rhs=bcol16[ot],
                                 start=(ot == 0), stop=(ot == 1))
            sbias4 = smallp.tile([128, 1], F32, tag="sbias")
            nc.scalar.copy(sbias4[:], sb_ps[:])
            SBIAS4[s] = sbias4

        def ph_xsel4(s):
            # xsel4 = 4 partition-copies of (W_sel @ x + S b'), via Wsel4.
            xsel4 = xselp.tile([128, PIX], F16, name="xsel4")
            for t in range(4):
                xt = ps.tile([128, 1024], F32, tag="s")
                for j in range(2):
                    w = t * 2 + j
                    for kt in range(2):
                        nc.tensor.matmul(
                            xt[:, j * 512:(j + 1) * 512],
                            lhsT=WSEL4[s][kt][:],
                            rhs=X16[s][kt][:, w * 512:(w + 1) * 512],
                            start=(kt == 0), stop=(kt == 1))
                nc.scalar.activation(xsel4[:, t * 1024:(t + 1) * 1024],
                                     xt[:], AF.Identity,
                                     bias=SBIAS4[s][:], scale=1.0)
            XSEL4[s] = xsel4
            if dbg:
                dma_out(out=dbg_xsel4[s * 128:(s + 1) * 128, :],
                        in_=xsel4[:])

        def ph_xsq4(s, eng):
            # xsq4 = xsel4^2 (fp16).  Only partitions 64:128 feed the Q
            # matmuls but computing all 128 costs the same (free-dim bound).
            xsq4 = xsqp.tile([128, PIX], F16, name="xsq4")
            if eng == "G":
                nc.gpsimd.tensor_tensor(xsq4[:], XSEL4[s][:], XSEL4[s][:],
                                        op=ALU.mult)
            else:
                nc.vector.tensor_tensor(xsq4[:], XSEL4[s][:], XSEL4[s][:],
                                        op=ALU.mult)
            XSQ4[s] = xsq4

        def ph_pair_half(s, m, half):
            # 4 windows of 512 via two 4-way row-tiled matmul quads:
            #   quad: S(w0)@rows0-31, S(w1)@rows32-63, Q(w0)@rows64-95,
            #         Q(w1)@rows96-127  (concurrent on the PE sub-arrays)
            # z = (Qneg + sh) + Square(S)
            p = min(128, CSE - m * 128)
            wp = wpair[m]
            xsel4 = XSEL4[s]
            xsq4 = XSQ4[s]
            if ZO[s][m] is None:
                ZO[s][m] = zp.tile([128, PIX], F16, name=f"zo{s}{m}",
                                   tag="zo")
            zo = ZO[s][m]
            sqT = sqp.tile([128, 2048], F16, tag="sq")
            qT = ps.tile([128, 2048], F32, tag="q", bufs=1)
            sAB = []
            for sub in range(2):                    # windows (w0,w1),(w2,w3)
                sA = ps.tile([128, 1024], F32, tag="s")
                w0 = half * 4 + sub * 2
                for j in range(2):
                    g = j                            # row groups 0,1 -> S
                    nc.tensor.matmul(
                        sA[0:p, j * 512:(j + 1) * 512],
                        lhsT=wp[32 * g:32 * (g + 1), 0:p],
                        rhs=xsel4[32 * g:32 * (g + 1),
                                  (w0 + j) * 512:(w0 + j + 1) * 512],
                        start=True, stop=True,
                        tile_position=(32 * g, 0))
                for j in range(2):
                    g = 2 + j                        # row groups 2,3 -> Qneg
                    nc.tensor.matmul(
                        qT[0:p, (sub * 2 + j) * 512:(sub * 2 + j + 1) * 512],
                        lhsT=wp[32 * g:32 * (g + 1), 0:p],
                        rhs=xsq4[32 * g:32 * (g + 1),
                                 (w0 + j) * 512:(w0 + j + 1) * 512],
                        start=True, stop=True,
                        tile_position=(32 * g, 0))
                sAB.append(sA)
            for sub in range(2):
                nc.scalar.activation(sqT[0:p, sub * 1024:(sub + 1) * 1024],
                                     sAB[sub][0:p, :], AF.Square)
            nc.vector.scalar_tensor_tensor(
                zo[0:p, half * 2048:(half + 1) * 2048],
                qT[0:p, :], shcol[m][0:p, :], sqT[0:p, :],
                op0=ALU.add, op1=ALU.add)
            if half == 1:
                r0 = s * COUT + C1 + m * 128
                dma_out(out=out_d[r0:r0 + p, :], in_=zo[0:p, :])

        def ph_y_unit(s, mt, t, eng):
            # one [128,1024] y unit: y = W'x + b' -> fp16 staging
            if YSB[s][mt] is None:
                YSB[s][mt] = ybp.tile([128, PIX], F16, name=f"ysb{s}{mt}",
                                      tag="ysb")
            ysb = YSB[s][mt]
            yt = ps.tile([128, 1024], F32, tag="s")
            for j in range(2):
                w = t * 2 + j
                for kt in range(2):
                    nc.tensor.matmul(
                        yt[:, j * 512:(j + 1) * 512],
                        lhsT=wyT16[kt][:, mt * 128:(mt + 1) * 128],
                        rhs=X16[s][kt][:, w * 512:(w + 1) * 512],
                        start=(kt == 0), stop=(kt == 1))
            dst = ysb[:, t * 1024:(t + 1) * 1024]
            if eng == "A":
                nc.scalar.activation(dst, yt[:], AF.Identity,
                                     bias=bcol[mt], scale=1.0)
            else:
                nc.vector.tensor_scalar(dst, yt[:], bcol[mt], None,
                                        op0=ALU.add)
            if t == 3:
                r0 = s * COUT + mt * 128
                dma_out(out=out_d[r0:r0 + 128, :], in_=ysb[:])

        # ---- emission schedule (per-engine FIFO order) ----
        ph_sel(0)
        ph_xsel4(0)
        ph_xsq4(0, "V")
        ph_sel(1)
        ph_y_unit(0, 0, 0, "A")
        ph_y_unit(0, 0, 1, "V")
        ph_pair_half(0, 0, 0)
        ph_pair_half(0, 0, 1)
        ph_y_unit(0, 0, 2, "A")
        ph_y_unit(0, 0, 3, "V")
        ph_pair_half(0, 1, 0)
        ph_pair_half(0, 1, 1)
        ph_xsel4(1)
        ph_xsq4(1, "G")
        ph_y_unit(0, 1, 0, "A")
        ph_y_unit(0, 1, 1, "V")
        ph_pair_half(0, 2, 0)
        ph_pair_half(0, 2, 1)
        ph_y_unit(0, 1, 2, "A")
        ph_y_unit(0, 1, 3, "V")
        ph_pair_half(0, 3, 0)
        ph_pair_half(0, 3, 1)
        ph_y_unit(1, 0, 0, "A")
        ph_y_unit(1, 0, 1, "V")
        ph_pair_half(1, 0, 0)
        ph_pair_half(1, 0, 1)
        ph_y_unit(1, 0, 2, "A")
        ph_y_unit(1, 0, 3, "V")
        ph_pair_half(1, 1, 0)
        ph_pair_half(1, 1, 1)
        ph_y_unit(1, 1, 0, "A")
        ph_y_unit(1, 1, 1, "V")
        ph_pair_half(1, 2, 0)
        ph_pair_half(1, 2, 1)
        ph_y_unit(1, 1, 2, "A")
        ph_y_unit(1, 1, 3, "V")
        ph_pair_half(1, 3, 0)
        ph_pair_half(1, 3, 1)


_CACHE = {}


def _get_nc(dbg: bool = False):
    key = f"nc{int(dbg)}"
    if key not in _CACHE:
        nc = bacc.Bacc("TRN2", target_bir_lowering=False, debug=False,
                       num_devices=NCORES)
        _build(nc, dbg=dbg)
        nc.compile()
        _CACHE[key] = nc
    return _CACHE[key]


def _host_params(w_fc, b_fc, g_x, b_x, m_x, v_x, eca_w, g_s, b_s, m_s, v_s):
    sx = (g_x / np.sqrt(v_x + EPS)).astype(np.float32)            # [256]
    Wp = (sx[:, None] * w_fc).astype(np.float32)                  # [o, c]
    bp = (sx * b_fc + b_x - m_x * sx).astype(np.float32)          # [256]

    cmat = np.zeros((C1, C1), np.float32)                         # [o, q]
    for k in range(5):
        d = k - 2                                                 # o - q
        for q in range(C1):
            o = q + d
            if 0 <= o < C1:
                cmat[o, q] = eca_w[k]

    tril = (np.arange(C1)[None, :] < np.arange(C1)[:, None]).astype(np.float32)
    offd = (1.0 - np.eye(C1, dtype=np.float32))

    hi, hj = np.triu_indices(CS, k=1)
    ss = (g_s / np.sqrt(v_s + EPS)).astype(np.float32)
    sh = (b_s - m_s * ss).astype(np.float32)
    # squares-trick pair weights:
    #   S = w16*(xi + xj), w16 = fp16(sqrt(ss/2))
    #   Qneg = -q16*(xi^2 + xj^2), q16 = fp16(w16^2)  (so S^2+Qneg ~= ss*xi*xj)
    w16 = np.sqrt(ss / 2.0).astype(np.float16)
    q16 = (w16.astype(np.float32) ** 2).astype(np.float16)
    wpair = np.zeros((MT4, 128, 128), np.float16)
    for m in range(MT4):
        p = min(128, CSE - m * 128)
        cols = np.arange(p)
        pq = m * 128 + cols
        blkS = np.zeros((32, 128), np.float32)
        blkS[hi[pq], cols] = w16[pq].astype(np.float32)
        blkS[hj[pq], cols] = w16[pq].astype(np.float32)
        blkQ = np.zeros((32, 128), np.float32)
        blkQ[hi[pq], cols] = -q16[pq].astype(np.float32)
        blkQ[hj[pq], cols] = -q16[pq].astype(np.float32)
        wpair[m][0:32] = blkS
        wpair[m][32:64] = blkS
        wpair[m][64:96] = blkQ
        wpair[m][96:128] = blkQ

    iota128 = np.tile(np.tile(np.arange(CS, dtype=np.float32), 4), (128, 1))

    return {
        "wyT16": Wp.T.astype(np.float16).copy(),
        "wyT32s": (Wp.T / PIX).astype(np.float32).copy(),
        "wfold16": Wp.astype(np.float16).copy(),
        "bcol": bp.reshape(C1, 1).copy(),
        "bcol16": bp.astype(np.float16).reshape(C1, 1).copy(),
        "cmat": cmat,
        "tril": tril,
        "offd": offd,
        "iota128": iota128,
        "wpair": wpair,
        "shcol": sh.reshape(CSE, 1).copy(),
    }


def _semantic_params(inputs):
    return _host_params(
        np.asarray(inputs["w_fc"], np.float32),
        np.asarray(inputs["b_fc"], np.float32),
        np.asarray(inputs["bn_x_gamma"], np.float32),
        np.asarray(inputs["bn_x_beta"], np.float32),
        np.asarray(inputs["bn_x_mean"], np.float32),
        np.asarray(inputs["bn_x_var"], np.float32),
        np.asarray(inputs["eca_w"], np.float32),
        np.asarray(inputs["bn_s_gamma"], np.float32),
        np.asarray(inputs["bn_s_beta"], np.float32),
        np.asarray(inputs["bn_s_mean"], np.float32),
        np.asarray(inputs["bn_s_var"], np.float32),
    )


def _pack_blobs(P):
    """Pack semantic params into the const blobs matching _build's layout."""
    cb32 = np.zeros((128, CB32), np.float32)
    for k in range(2):
        cb32[:, _WY32 + k * 256: _WY32 + (k + 1) * 256] = \
            P["wyT32s"][k * 128:(k + 1) * 128]
        cb32[:, _CMAT + k * 256: _CMAT + (k + 1) * 256] = \
            P["cmat"][k * 128:(k + 1) * 128]
        cb32[:, _TRIL + k * 256: _TRIL + (k + 1) * 256] = \
            P["tril"][k * 128:(k + 1) * 128]
        cb32[:, _OFFD + k * 256: _OFFD + (k + 1) * 256] = \
            P["offd"][k * 128:(k + 1) * 128]
        cb32[:, _BCOL + k] = P["bcol"][k * 128:(k + 1) * 128, 0]
    for m in range(MT4):
        p = min(128, CSE - m * 128)
        cb32[:p, _SHCOL + m] = P["shcol"][m * 128: m * 128 + p, 0]
    cb32[:, _IOTA:_IOTA + 128] = P["iota128"]

    cb16 = np.zeros((128, CB16), np.float16)
    for k in range(2):
        cb16[:, _WY16 + k * 256: _WY16 + (k + 1) * 256] = \
            P["wyT16"][k * 128:(k + 1) * 128]
        cb16[:, _WFOLD + k * 256: _WFOLD + (k + 1) * 256] = \
            P["wfold16"][k * 128:(k + 1) * 128]
        cb16[:, _BC16 + k] = P["bcol16"][k * 128:(k + 1) * 128, 0]
    for m in range(MT4):
        cb16[:, _WPAIR + m * 128: _WPAIR + (m + 1) * 128] = P["wpair"][m]

    return {"cb32": cb32, "cb16": np.ascontiguousarray(cb16)}


def _in_maps(inputs):
    x = np.ascontiguousarray(np.asarray(inputs["x"], np.float32))
    blobs = _pack_blobs(_semantic_params(inputs))
    maps = []
    for c in range(NCORES):
        shard = x[c * SPC:(c + 1) * SPC].reshape(SPC * C1, PIX)
        # exact f32 per-channel pixel sums (feeds the pooled/top-k path);
        # fp16 x feeds the matmuls
        xsum = shard.sum(axis=1, dtype=np.float32)          # [512]
        xsumv = np.zeros((128, 2 * SPC), np.float32)
        for s in range(SPC):
            for kt in range(2):
                xsumv[:, s * 2 + kt] = xsum[s * C1 + kt * 128:
                                            s * C1 + (kt + 1) * 128]
        maps.append({"x16v": shard.astype(np.float16),
                     "xsumv": xsumv, **blobs})
    return maps


def _ensure_ntff_hook():
    """The agent image lacks antenv.axon_hooks; synthesize it so
    run_bass_kernel_spmd(trace=True) can reach the NTFF profiler in
    libaxon_pjrt.so. Safe no-op if anything is missing."""
    try:
        import antenv.axon_hooks  # noqa: F401
        return
    except ImportError:
        pass
    try:
        import types
        import antenv
        from trn_agent_boot.trn_boot import _ntff_profile_via_ctypes
        hook = _ntff_profile_via_ctypes("/opt/axon/libaxon_pjrt.so")
        mod = types.ModuleType("antenv.axon_hooks")
        mod._hook = hook
        mod.get_axon_ntff_profile_hook = lambda: mod._hook
        mod.set_axon_ntff_profile_hook = lambda h: setattr(mod, "_hook", h)
        sys.modules["antenv.axon_hooks"] = mod
        antenv.axon_hooks = mod
    except Exception as e:  # pragma: no cover
        print(f"ntff hook shim failed: {e}", file=sys.stderr)


def run(inputs, trace=False, dbg=False):
    if trace:
        _ensure_ntff_hook()
    nc = _get_nc(dbg=dbg)
    maps = _in_maps(inputs)
    res = run_bass_kernel_spmd(nc, maps, core_ids=list(range(NCORES)),
                               trace=trace)
    outs = [np.asarray(res.results[c]["out"]).astype(np.float32)
            .reshape(SPC, COUT, H, W) for c in range(NCORES)]
    return np.concatenate(outs, axis=0), res


def kernel(**inputs) -> np.ndarray:
    out, _ = run(inputs, trace=False)
    return out


# revision 16
# speedup vs baseline: 1.2136x; 1.1885x over previous
# Trainium2 Bass kernel for nn_AdaptiveCrossHadamard (v2).
#
# Reference computation (per sample):
#   y   = BN(Conv1x1(x))                                  [256, 64*64]
#   p   = mean_pixels(y); logits = conv1d(p, eca_w, k=5)  [256]
#   idx = top_32(logits) (sorted desc, ties -> lower idx)
#   xs  = y[idx]                                          [32, 4096]
#   z   = BN_s(xs[hi] * xs[hj])  for all i<j pairs        [496, 4096]
#   out = concat([y, z], channel axis)                    [752, 4096]
#
# v2 strategy (8 NeuronCores, batch-parallel, 2 samples/core):
#   - fp16 output staging + DMA (halves HBM write traffic); host casts to f32.
#   - xsel computed 4x-replicated across partition groups (Wsel4 weights make
#     the PE produce 4 copies for free); xsq4 = xsel4^2 on DVE/GPSIMD.
#   - pair phase: per 128-row tile m a composite [128,128] weight holds the
#     S one-hot (sum) block twice (rows 0-63) and the negated Q one-hot
#     (square-sum) block twice (rows 64-127).  Four row-tiled matmuls
#     (tile_position) run concurrently: two S windows + two Q windows per
#     span, writing [128,1024] S tiles and a [128,2048] Q tile.
#   - z = (Qneg + sh) + S^2: ACT does Square(S)->fp16, DVE does one
#     scalar_tensor_tensor per [128,2048].
#   - y bias-move split between ACT and DVE (tunable) to balance engines.
#   - input DMAs ride the scalar HWDGE queue; outputs own the sync queue.
import os
import sys
import numpy as np

_TRN_REPO = "/opt/trn_rl_repo"
if _TRN_REPO not in sys.path and os.path.isdir(_TRN_REPO):
    sys.path.insert(0, _TRN_REPO)

import concourse.bacc as bacc
import concourse.bass as bass
import concourse.mybir as mybir
import concourse.tile as tile
from concourse.bass_utils import run_bass_kernel_spmd

F32 = mybir.dt.float32
F16 = mybir.dt.float16
AF = mybir.ActivationFunctionType
ALU = mybir.AluOpType

B, C1, H, W = 16, 256, 64, 64
PIX = H * W                      # 4096
CS = 32
CSE = CS * (CS - 1) // 2         # 496
NCORES = 8
SPC = B // NCORES                # samples per core = 2
COUT = C1 + CSE                  # 752
EPS = 1e-5
MT4 = (CSE + 127) // 128         # 4 pair-row tiles (128,128,128,112)

# f32 const blob column layout: [128, CB32]
_WY32 = 0                        # wyT32s, 2 chunks x 256
_CMAT = 512                      # cmat, 2 x 256
_TRIL = 1024                     # tril, 2 x 256
_OFFD = 1536                     # offd (1 - eye), 2 x 256
_BCOL = 2048                     # bcol, 2 x 1
_SHCOL = 2050                    # shift_s per m-tile, 4 x 1
_IOTA = 2054                     # iota128 (j % 32), 128
CB32 = 2182
# fp16 const blob: [128, CB16]
_WY16 = 0                        # wyT16, 2 x 256
_WFOLD = 512                     # wfold16, 2 x 256
_BC16 = 1024                     # bcol16, 2 x 1
_WPAIR = 1026                    # pair composites, 4 x 128
CB16 = 2050


def _build(nc: bass.Bass, dbg: bool = False):
    """Emit the per-core Tile program. SPMD: all 8 cores run this graph."""
    x_d = nc.dram_tensor("x16v", [SPC * C1, PIX], F16, kind="ExternalInput")
    xsum_d = nc.dram_tensor("xsumv", [128, 2 * SPC], F32, kind="ExternalInput")
    out_d = nc.dram_tensor("out", [SPC * COUT, PIX], F16, kind="ExternalOutput")
    cb32_d = nc.dram_tensor("cb32", [128, CB32], F32, kind="ExternalInput")
    cb16_d = nc.dram_tensor("cb16", [128, CB16], F16, kind="ExternalInput")
    if dbg:
        dbg_rank = nc.dram_tensor("dbg_rank", [SPC * C1, 1], F32,
                                  kind="ExternalOutput")
        dbg_xsel4 = nc.dram_tensor("dbg_xsel4", [SPC * 128, PIX], F16,
                                   kind="ExternalOutput")

    from contextlib import ExitStack
    with tile.TileContext(nc) as tc, ExitStack() as ctx:
        cpool = ctx.enter_context(tc.tile_pool(name="consts", bufs=1))
        x16p = ctx.enter_context(tc.tile_pool(name="x16", bufs=4))
        xselp = ctx.enter_context(tc.tile_pool(name="xsel4", bufs=2))
        xsqp = ctx.enter_context(tc.tile_pool(name="xsq4", bufs=2))
        sqp = ctx.enter_context(tc.tile_pool(name="sq16", bufs=2))
        zp = ctx.enter_context(tc.tile_pool(name="zout", bufs=3))
        ybp = ctx.enter_context(tc.tile_pool(name="ysb", bufs=2))
        gp = ctx.enter_context(tc.tile_pool(name="gwork", bufs=2))
        smallp = ctx.enter_context(tc.tile_pool(name="small", bufs=4))
        # PSUM: tag "s" = [128,1024] f32 (2 banks) x2 bufs; tag "q" =
        # [128,2048] f32 (4 banks) x1 buf.  8 banks total.
        ps = ctx.enter_context(tc.tile_pool(name="ps", bufs=2, space="PSUM"))

        dma_out = nc.sync.dma_start          # outputs own the sync queue
        dma_in = nc.scalar.dma_start         # inputs on the ACT HWDGE queue

        # ---- constants first on the input queue (they gate the top-k
        # chain), then x tiles ----
        xsumt = cpool.tile([128, 2 * SPC], F32, tag="xsumt")
        dma_in(out=xsumt[:], in_=xsum_d[:, :])
        cb32 = cpool.tile([128, CB32], F32, tag="cb32")
        dma_in(out=cb32[:], in_=cb32_d[:, :])
        cb16 = cpool.tile([128, CB16], F16, tag="cb16")
        dma_in(out=cb16[:], in_=cb16_d[:, :])

        # HAM warmup fodder: a zeroed fp16 tile the PE chews on during the
        # x-load dead time so the clock gate flips to 8/8 before real work.
        warm = cpool.tile([128, 512], F16, tag="warm")
        nc.vector.memset(warm[:], 0.0)
        warm_ps = ps.tile([128, 2048], F32, tag="q", bufs=1, name="warm_ps")

        def ph_warm(n):
            for i in range(n):
                nc.tensor.matmul(warm_ps[:, (i % 4) * 512:(i % 4 + 1) * 512],
                                 lhsT=warm[:, 0:128], rhs=warm[:, 0:512],
                                 start=True, stop=True)

        HPIX = PIX // 2
        X16 = [[None] * 2 for _ in range(SPC)]
        for s in range(SPC):
            for kt in range(2):
                xt = x16p.tile([128, PIX], F16, name="xt")
                r0 = s * C1 + kt * 128
                dma_in(out=xt[:, :HPIX], in_=x_d[r0:r0 + 128, :HPIX])
                dma_in(out=xt[:, HPIX:], in_=x_d[r0:r0 + 128, HPIX:])
                X16[s][kt] = xt

        def c32(col, w):
            return cb32[:, col:col + w]

        wyT32s = [c32(_WY32 + k * 256, 256) for k in range(2)]
        cmat = [c32(_CMAT + k * 256, 256) for k in range(2)]
        tril = [c32(_TRIL + k * 256, 256) for k in range(2)]
        offd = [c32(_OFFD + k * 256, 256) for k in range(2)]
        bcol = [c32(_BCOL + k, 1) for k in range(2)]
        shcol = [c32(_SHCOL + m, 1) for m in range(MT4)]
        iota128 = c32(_IOTA, 128)
        wyT16 = [cb16[:, _WY16 + k * 256: _WY16 + (k + 1) * 256]
                 for k in range(2)]
        wfold16 = [cb16[:, _WFOLD + k * 256: _WFOLD + (k + 1) * 256]
                   for k in range(2)]
        bcol16 = [cb16[:, _BC16 + k: _BC16 + k + 1] for k in range(2)]
        wpair = [cb16[:, _WPAIR + m * 128: _WPAIR + (m + 1) * 128]
                 for m in range(MT4)]

        WSEL4 = [None] * SPC
        SBIAS4 = [None] * SPC
        XSEL4 = [None] * SPC
        XSQ4 = [None] * SPC
        YSB = [[None] * 2 for _ in range(SPC)]
        ZO = [[None] * MT4 for _ in range(SPC)]

        def ph_sel(s):
            # pooled = W'@xbar + b' (exact f32; wyT32s folds the /4096)
            pooled = []
            for mt in range(2):
                pp = ps.tile([128, 1], F32, tag="s")
                for kt in range(2):
                    nc.tensor.matmul(
                        pp[:], lhsT=wyT32s[kt][:, mt * 128:(mt + 1) * 128],
                        rhs=xsumt[:, s * 2 + kt: s * 2 + kt + 1],
                        start=(kt == 0), stop=(kt == 1))
                pb = smallp.tile([128, 1], F32, tag="pooled")
                nc.scalar.activation(pb[:], pp[:], AF.Identity,
                                     bias=bcol[mt], scale=1.0)
                pooled.append(pb)

            lr_ps = ps.tile([1, C1], F32, tag="s")
            for ot in range(2):
                nc.tensor.matmul(lr_ps[:], lhsT=pooled[ot][:], rhs=cmat[ot],
                                 start=(ot == 0), stop=(ot == 1))
            lrow = smallp.tile([1, C1], F32, tag="lrow")
            nc.scalar.copy(lrow[:], lr_ps[:])

            st4 = []
            for qt in range(2):
                lc_ps = ps.tile([128, 1], F32, tag="s")
                for ot in range(2):
                    nc.tensor.matmul(
                        lc_ps[:], lhsT=cmat[ot][:, qt * 128:(qt + 1) * 128],
                        rhs=pooled[ot][:], start=(ot == 0), stop=(ot == 1))
                lcol = smallp.tile([128, 1], F32, tag="lcol")
                nc.scalar.copy(lcol[:], lc_ps[:])

                # exact broadcast of logits row to all partitions
                brow = gp.tile([128, C1], F32, tag="brow")
                nc.gpsimd.partition_broadcast(brow[:], lrow[:])
                # rank[a] = #{b!=a: logits[b] > logits[a]}
                #        + #{b < a: logits[b] == logits[a]}   (jax tie-break)
                # offd kills the diagonal: lrow/lcol come from different
                # matmuls so brow[a,a] vs lcol[a] can misfire by 1 ulp.
                g2 = gp.tile([128, C1], F32)
                nc.vector.scalar_tensor_tensor(
                    g2[:], brow[:], lcol[:], tril[qt],
                    op0=ALU.is_equal, op1=ALU.mult)
                gsum = gp.tile([128, C1], F32)
                nc.vector.scalar_tensor_tensor(
                    gsum[:], brow[:], lcol[:], g2[:],
                    op0=ALU.is_gt, op1=ALU.add)
                gm = gp.tile([128, C1], F32)
                nc.vector.tensor_tensor(gm[:], gsum[:], offd[qt], op=ALU.mult)
                rank = smallp.tile([128, 1], F32, tag="rank")
                nc.vector.tensor_reduce(rank[:], gm[:],
                                        axis=mybir.AxisListType.X, op=ALU.add)
                # S4_T[c, 32g+k] = (rank[c] == k), 4 horizontal copies
                stq = smallp.tile([128, 128], F16, tag="st", bufs=4)
                nc.vector.tensor_scalar(stq[:], iota128, rank[:], None,
                                        op0=ALU.is_equal)
                st4.append(stq)
                if dbg:
                    r0 = s * C1 + qt * 128
                    dma_out(out=dbg_rank[r0:r0 + 128, :], in_=rank[:])

            # selection weights (4 copies): W_sel4T[c, 32g+k]
            wsel4 = []
            for ct in range(2):
                ws_ps = ps.tile([128, 128], F32, tag="s")
                for ot in range(2):
                    nc.tensor.matmul(
                        ws_ps[:], lhsT=wfold16[ot][:, ct * 128:(ct + 1) * 128],
                        rhs=st4[ot][:], start=(ot == 0), stop=(ot == 1))
                wsq = smallp.tile([128, 128], F16, tag="wsel", bufs=4)
                nc.scalar.copy(wsq[:], ws_ps[:])
                wsel4.append(wsq)
            WSEL4[s] = wsel4
            sb_ps = ps.tile([128, 1], F32, tag="s")
            for ot in range(2):
                nc.tensor.matmul(sb_ps[:], lhsT=st4[ot][:], rhs=bcol16[ot],
                                 start=(ot == 0), stop=(ot == 1))
            sbias4 = smallp.tile([128, 1], F32, tag="sbias")
            nc.scalar.copy(sbias4[:], sb_ps[:])
            SBIAS4[s] = sbias4

        def ph_xsel4(s):
            # xsel4 = 4 partition-copies of (W_sel @ x + S b'), via Wsel4.
            xsel4 = xselp.tile([128, PIX], F16, name="xsel4")
            for t in range(4):
                xt = ps.tile([128, 1024], F32, tag="s")
                for kt in range(2):          # same weights consecutive
                    for j in range(2):
                        w = t * 2 + j
                        nc.tensor.matmul(
                            xt[:, j * 512:(j + 1) * 512],
                            lhsT=WSEL4[s][kt][:],
                            rhs=X16[s][kt][:, w * 512:(w + 1) * 512],
                            start=(kt == 0), stop=(kt == 1))
                nc.scalar.activation(xsel4[:, t * 1024:(t + 1) * 1024],
                                     xt[:], AF.Identity,
                                     bias=SBIAS4[s][:], scale=1.0)
            XSEL4[s] = xsel4
            if dbg:
                dma_out(out=dbg_xsel4[s * 128:(s + 1) * 128, :],
                        in_=xsel4[:])

        def ph_xsq4(s, eng):
            # xsq4 = xsel4^2 (fp16).  Only partitions 64:128 feed the Q
            # matmuls but computing all 128 costs the same (free-dim bound).
            xsq4 = xsqp.tile([128, PIX], F16, name="xsq4")
            if eng == "G":
                nc.gpsimd.tensor_tensor(xsq4[:], XSEL4[s][:], XSEL4[s][:],
                                        op=ALU.mult)
            else:
                nc.vector.tensor_tensor(xsq4[:], XSEL4[s][:], XSEL4[s][:],
                                        op=ALU.mult)
            XSQ4[s] = xsq4

        def ph_pair_half(s, m, half):
            # 4 windows of 512 via two 4-way row-tiled matmul quads:
            #   quad: S(w0)@rows0-31, S(w1)@rows32-63, Q(w0)@rows64-95,
            #         Q(w1)@rows96-127  (concurrent on the PE sub-arrays)
            # z = (Qneg + sh) + Square(S)
            p = min(128, CSE - m * 128)
            wp = wpair[m]
            xsel4 = XSEL4[s]
            xsq4 = XSQ4[s]
            if ZO[s][m] is None:
                ZO[s][m] = zp.tile([128, PIX], F16, name=f"zo{s}{m}",
                                   tag="zo")
            zo = ZO[s][m]
            sqT = sqp.tile([128, 2048], F16, tag="sq")
            qT = ps.tile([128, 2048], F32, tag="q", bufs=1)
            sAB = []
            for sub in range(2):                    # windows (w0,w1),(w2,w3)
                sA = ps.tile([128, 1024], F32, tag="s")
                w0 = half * 4 + sub * 2
                for j in range(2):
                    g = j                            # row groups 0,1 -> S
                    nc.tensor.matmul(
                        sA[0:p, j * 512:(j + 1) * 512],
                        lhsT=wp[32 * g:32 * (g + 1), 0:p],
                        rhs=xsel4[32 * g:32 * (g + 1),
                                  (w0 + j) * 512:(w0 + j + 1) * 512],
                        start=True, stop=True,
                        tile_position=(32 * g, 0))
                for j in range(2):
                    g = 2 + j                        # row groups 2,3 -> Qneg
                    nc.tensor.matmul(
                        qT[0:p, (sub * 2 + j) * 512:(sub * 2 + j + 1) * 512],
                        lhsT=wp[32 * g:32 * (g + 1), 0:p],
                        rhs=xsq4[32 * g:32 * (g + 1),
                                 (w0 + j) * 512:(w0 + j + 1) * 512],
                        start=True, stop=True,
                        tile_position=(32 * g, 0))
                sAB.append(sA)
            for sub in range(2):
                nc.scalar.activation(sqT[0:p, sub * 1024:(sub + 1) * 1024],
                                     sAB[sub][0:p, :], AF.Square)
            nc.vector.scalar_tensor_tensor(
                zo[0:p, half * 2048:(half + 1) * 2048],
                qT[0:p, :], shcol[m][0:p, :], sqT[0:p, :],
                op0=ALU.add, op1=ALU.add)
            if half == 1:
                r0 = s * COUT + C1 + m * 128
                dma_out(out=out_d[r0:r0 + p, :], in_=zo[0:p, :])

        def ph_y_unit(s, mt, t, eng):
            # one [128,1024] y unit: y = W'x + b' -> fp16 staging
            if YSB[s][mt] is None:
                YSB[s][mt] = ybp.tile([128, PIX], F16, name=f"ysb{s}{mt}",
                                      tag="ysb")
            ysb = YSB[s][mt]
            yt = ps.tile([128, 1024], F32, tag="s")
            for kt in range(2):              # same weights consecutive
                for j in range(2):
                    w = t * 2 + j
                    nc.tensor.matmul(
                        yt[:, j * 512:(j + 1) * 512],
                        lhsT=wyT16[kt][:, mt * 128:(mt + 1) * 128],
                        rhs=X16[s][kt][:, w * 512:(w + 1) * 512],
                        start=(kt == 0), stop=(kt == 1))
            dst = ysb[:, t * 1024:(t + 1) * 1024]
            if eng == "A":
                nc.scalar.activation(dst, yt[:], AF.Identity,
                                     bias=bcol[mt], scale=1.0)
            else:
                nc.vector.tensor_scalar(dst, yt[:], bcol[mt], None,
                                        op0=ALU.add)
            if t == 3:
                r0 = s * COUT + mt * 128
                dma_out(out=out_d[r0:r0 + 128, :], in_=ysb[:])

        # ---- emission schedule (per-engine FIFO order) ----
        ph_warm(14)
        ph_sel(0)
        ph_warm(6)
        ph_xsel4(0)
        ph_xsq4(0, "V")
        ph_sel(1)
        ph_warm(6)
        ph_y_unit(0, 0, 0, "A")
        ph_y_unit(0, 0, 1, "V")
        ph_pair_half(0, 0, 0)
        ph_pair_half(0, 0, 1)
        ph_y_unit(0, 0, 2, "A")
        ph_y_unit(0, 0, 3, "V")
        ph_pair_half(0, 1, 0)
        ph_pair_half(0, 1, 1)
        ph_xsel4(1)
        ph_xsq4(1, "G")
        ph_y_unit(0, 1, 0, "A")
        ph_y_unit(0, 1, 1, "V")
        ph_pair_half(0, 2, 0)
        ph_pair_half(0, 2, 1)
        ph_y_unit(0, 1, 2, "A")
        ph_y_unit(0, 1, 3, "V")
        ph_pair_half(0, 3, 0)
        ph_pair_half(0, 3, 1)
        ph_y_unit(1, 0, 0, "A")
        ph_y_unit(1, 0, 1, "V")
        ph_pair_half(1, 0, 0)
        ph_pair_half(1, 0, 1)
        ph_y_unit(1, 0, 2, "A")
        ph_y_unit(1, 0, 3, "V")
        ph_pair_half(1, 1, 0)
        ph_pair_half(1, 1, 1)
        ph_y_unit(1, 1, 0, "A")
        ph_y_unit(1, 1, 1, "V")
        ph_pair_half(1, 2, 0)
        ph_pair_half(1, 2, 1)
        ph_y_unit(1, 1, 2, "A")
        ph_y_unit(1, 1, 3, "V")
        ph_pair_half(1, 3, 0)
        ph_pair_half(1, 3, 1)


_CACHE = {}


def _get_nc(dbg: bool = False):
    key = f"nc{int(dbg)}"
    if key not in _CACHE:
        nc = bacc.Bacc("TRN2", target_bir_lowering=False, debug=False,
                       num_devices=NCORES)
        _build(nc, dbg=dbg)
        nc.compile()
        _CACHE[key] = nc
    return _CACHE[key]


def _host_params(w_fc, b_fc, g_x, b_x, m_x, v_x, eca_w, g_s, b_s, m_s, v_s):
    sx = (g_x / np.sqrt(v_x + EPS)).astype(np.float32)            # [256]
    Wp = (sx[:, None] * w_fc).astype(np.float32)                  # [o, c]
    bp = (sx * b_fc + b_x - m_x * sx).astype(np.float32)          # [256]

    cmat = np.zeros((C1, C1), np.float32)                         # [o, q]
    for k in range(5):
        d = k - 2                                                 # o - q
        for q in range(C1):
            o = q + d
            if 0 <= o < C1:
                cmat[o, q] = eca_w[k]

    tril = (np.arange(C1)[None, :] < np.arange(C1)[:, None]).astype(np.float32)
    offd = (1.0 - np.eye(C1, dtype=np.float32))

    hi, hj = np.triu_indices(CS, k=1)
    ss = (g_s / np.sqrt(v_s + EPS)).astype(np.float32)
    sh = (b_s - m_s * ss).astype(np.float32)
    # squares-trick pair weights:
    #   S = w16*(xi + xj), w16 = fp16(sqrt(ss/2))
    #   Qneg = -q16*(xi^2 + xj^2), q16 = fp16(w16^2)  (so S^2+Qneg ~= ss*xi*xj)
    w16 = np.sqrt(ss / 2.0).astype(np.float16)
    q16 = (w16.astype(np.float32) ** 2).astype(np.float16)
    wpair = np.zeros((MT4, 128, 128), np.float16)
    for m in range(MT4):
        p = min(128, CSE - m * 128)
        cols = np.arange(p)
        pq = m * 128 + cols
        blkS = np.zeros((32, 128), np.float32)
        blkS[hi[pq], cols] = w16[pq].astype(np.float32)
        blkS[hj[pq], cols] = w16[pq].astype(np.float32)
        blkQ = np.zeros((32, 128), np.float32)
        blkQ[hi[pq], cols] = -q16[pq].astype(np.float32)
        blkQ[hj[pq], cols] = -q16[pq].astype(np.float32)
        wpair[m][0:32] = blkS
        wpair[m][32:64] = blkS
        wpair[m][64:96] = blkQ
        wpair[m][96:128] = blkQ

    iota128 = np.tile(np.tile(np.arange(CS, dtype=np.float32), 4), (128, 1))

    return {
        "wyT16": Wp.T.astype(np.float16).copy(),
        "wyT32s": (Wp.T / PIX).astype(np.float32).copy(),
        "wfold16": Wp.astype(np.float16).copy(),
        "bcol": bp.reshape(C1, 1).copy(),
        "bcol16": bp.astype(np.float16).reshape(C1, 1).copy(),
        "cmat": cmat,
        "tril": tril,
        "offd": offd,
        "iota128": iota128,
        "wpair": wpair,
        "shcol": sh.reshape(CSE, 1).copy(),
    }


def _semantic_params(inputs):
    return _host_params(
        np.asarray(inputs["w_fc"], np.float32),
        np.asarray(inputs["b_fc"], np.float32),
        np.asarray(inputs["bn_x_gamma"], np.float32),
        np.asarray(inputs["bn_x_beta"], np.float32),
        np.asarray(inputs["bn_x_mean"], np.float32),
        np.asarray(inputs["bn_x_var"], np.float32),
        np.asarray(inputs["eca_w"], np.float32),
        np.asarray(inputs["bn_s_gamma"], np.float32),
        np.asarray(inputs["bn_s_beta"], np.float32),
        np.asarray(inputs["bn_s_mean"], np.float32),
        np.asarray(inputs["bn_s_var"], np.float32),
    )


def _pack_blobs(P):
    """Pack semantic params into the const blobs matching _build's layout."""
    cb32 = np.zeros((128, CB32), np.float32)
    for k in range(2):
        cb32[:, _WY32 + k * 256: _WY32 + (k + 1) * 256] = \
            P["wyT32s"][k * 128:(k + 1) * 128]
        cb32[:, _CMAT + k * 256: _CMAT + (k + 1) * 256] = \
            P["cmat"][k * 128:(k + 1) * 128]
        cb32[:, _TRIL + k * 256: _TRIL + (k + 1) * 256] = \
            P["tril"][k * 128:(k + 1) * 128]
        cb32[:, _OFFD + k * 256: _OFFD + (k + 1) * 256] = \
            P["offd"][k * 128:(k + 1) * 128]
        cb32[:, _BCOL + k] = P["bcol"][k * 128:(k + 1) * 128, 0]
    for m in range(MT4):
        p = min(128, CSE - m * 128)
        cb32[:p, _SHCOL + m] = P["shcol"][m * 128: m * 128 + p, 0]
    cb32[:, _IOTA:_IOTA + 128] = P["iota128"]

    cb16 = np.zeros((128, CB16), np.float16)
    for k in range(2):
        cb16[:, _WY16 + k * 256: _WY16 + (k + 1) * 256] = \
            P["wyT16"][k * 128:(k + 1) * 128]
        cb16[:, _WFOLD + k * 256: _WFOLD + (k + 1) * 256] = \
            P["wfold16"][k * 128:(k + 1) * 128]
        cb16[:, _BC16 + k] = P["bcol16"][k * 128:(k + 1) * 128, 0]
    for m in range(MT4):
        cb16[:, _WPAIR + m * 128: _WPAIR + (m + 1) * 128] = P["wpair"][m]

    return {"cb32": cb32, "cb16": np.ascontiguousarray(cb16)}


def _in_maps(inputs):
    x = np.ascontiguousarray(np.asarray(inputs["x"], np.float32))
    blobs = _pack_blobs(_semantic_params(inputs))
    maps = []
    for c in range(NCORES):
        shard = x[c * SPC:(c + 1) * SPC].reshape(SPC * C1, PIX)
        # exact f32 per-channel pixel sums (feeds the pooled/top-k path);
        # fp16 x feeds the matmuls
        xsum = shard.sum(axis=1, dtype=np.float32)          # [512]
        xsumv = np.zeros((128, 2 * SPC), np.float32)
        for s in range(SPC):
            for kt in range(2):
                xsumv[:, s * 2 + kt] = xsum[s * C1 + kt * 128:
                                            s * C1 + (kt + 1) * 128]
        maps.append({"x16v": shard.astype(np.float16),
                     "xsumv": xsumv, **blobs})
    return maps


def _ensure_ntff_hook():
    """The agent image lacks antenv.axon_hooks; synthesize it so
    run_bass_kernel_spmd(trace=True) can reach the NTFF profiler in
    libaxon_pjrt.so. Safe no-op if anything is missing."""
    try:
        import antenv.axon_hooks  # noqa: F401
        return
    except ImportError:
        pass
    try:
        import types
        import antenv
        from trn_agent_boot.trn_boot import _ntff_profile_via_ctypes
        hook = _ntff_profile_via_ctypes("/opt/axon/libaxon_pjrt.so")
        mod = types.ModuleType("antenv.axon_hooks")
        mod._hook = hook
        mod.get_axon_ntff_profile_hook = lambda: mod._hook
        mod.set_axon_ntff_profile_hook = lambda h: setattr(mod, "_hook", h)
        sys.modules["antenv.axon_hooks"] = mod
        antenv.axon_hooks = mod
    except Exception as e:  # pragma: no cover
        print(f"ntff hook shim failed: {e}", file=sys.stderr)


def run(inputs, trace=False, dbg=False):
    if trace:
        _ensure_ntff_hook()
    nc = _get_nc(dbg=dbg)
    maps = _in_maps(inputs)
    res = run_bass_kernel_spmd(nc, maps, core_ids=list(range(NCORES)),
                               trace=trace)
    outs = [np.asarray(res.results[c]["out"]).astype(np.float32)
            .reshape(SPC, COUT, H, W) for c in range(NCORES)]
    return np.concatenate(outs, axis=0), res


def kernel(**inputs) -> np.ndarray:
    out, _ = run(inputs, trace=False)
    return out


# revision 19
# speedup vs baseline: 1.2644x; 1.0418x over previous
# Trainium2 Bass kernel for nn_AdaptiveCrossHadamard (v2).
#
# Reference computation (per sample):
#   y   = BN(Conv1x1(x))                                  [256, 64*64]
#   p   = mean_pixels(y); logits = conv1d(p, eca_w, k=5)  [256]
#   idx = top_32(logits) (sorted desc, ties -> lower idx)
#   xs  = y[idx]                                          [32, 4096]
#   z   = BN_s(xs[hi] * xs[hj])  for all i<j pairs        [496, 4096]
#   out = concat([y, z], channel axis)                    [752, 4096]
#
# v2 strategy (8 NeuronCores, batch-parallel, 2 samples/core):
#   - fp16 output staging + DMA (halves HBM write traffic); host casts to f32.
#   - xsel computed 4x-replicated across partition groups (Wsel4 weights make
#     the PE produce 4 copies for free); xsq4 = xsel4^2 on DVE/GPSIMD.
#   - pair phase: per 128-row tile m a composite [128,128] weight holds the
#     S one-hot (sum) block twice (rows 0-63) and the negated Q one-hot
#     (square-sum) block twice (rows 64-127).  Four row-tiled matmuls
#     (tile_position) run concurrently: two S windows + two Q windows per
#     span, writing [128,1024] S tiles and a [128,2048] Q tile.
#   - z = (Qneg + sh) + S^2: ACT does Square(S)->fp16, DVE does one
#     scalar_tensor_tensor per [128,2048].
#   - y bias-move split between ACT and DVE (tunable) to balance engines.
#   - input DMAs ride the scalar HWDGE queue; outputs own the sync queue.
import os
import sys
import numpy as np

_TRN_REPO = "/opt/trn_rl_repo"
if _TRN_REPO not in sys.path and os.path.isdir(_TRN_REPO):
    sys.path.insert(0, _TRN_REPO)

import concourse.bacc as bacc
import concourse.bass as bass
import concourse.mybir as mybir
import concourse.tile as tile
from concourse.bass_utils import run_bass_kernel_spmd

F32 = mybir.dt.float32
F16 = mybir.dt.float16
AF = mybir.ActivationFunctionType
ALU = mybir.AluOpType

B, C1, H, W = 16, 256, 64, 64
PIX = H * W                      # 4096
CS = 32
CSE = CS * (CS - 1) // 2         # 496
NCORES = 8
SPC = B // NCORES                # samples per core = 2
COUT = C1 + CSE                  # 752
EPS = 1e-5
MT4 = (CSE + 127) // 128         # 4 pair-row tiles (128,128,128,112)

# f32 const blob column layout: [128, CB32]
_WY32 = 0                        # wyT32s, 2 chunks x 256
_CMAT = 512                      # cmat, 2 x 256
_TRIL = 1024                     # tril, 2 x 256
_OFFD = 1536                     # offd (1 - eye), 2 x 256
_BCOL = 2048                     # bcol, 2 x 1
_SHCOL = 2050                    # shift_s per m-tile, 4 x 1
_IOTA = 2054                     # iota128 (j % 32), 128
CB32 = 2182
# fp16 const blob: [128, CB16]
_WY16 = 0                        # wyT16, 2 x 256
_WFOLD = 512                     # wfold16, 2 x 256
_BC16 = 1024                     # bcol16, 2 x 1
_WPAIR = 1026                    # pair composites, 4 x 128
CB16 = 2050


def _build(nc: bass.Bass, dbg: bool = False):
    """Emit the per-core Tile program. SPMD: all 8 cores run this graph."""
    x_d = nc.dram_tensor("x16v", [SPC * C1, PIX], F16, kind="ExternalInput")
    xsum_d = nc.dram_tensor("xsumv", [128, 2 * SPC], F32, kind="ExternalInput")
    out_d = nc.dram_tensor("out", [SPC * COUT, PIX], F16, kind="ExternalOutput")
    cb32_d = nc.dram_tensor("cb32", [128, CB32], F32, kind="ExternalInput")
    cb16_d = nc.dram_tensor("cb16", [128, CB16], F16, kind="ExternalInput")
    if dbg:
        dbg_rank = nc.dram_tensor("dbg_rank", [SPC * C1, 1], F32,
                                  kind="ExternalOutput")
        dbg_xsel4 = nc.dram_tensor("dbg_xsel4", [SPC * 128, PIX], F16,
                                   kind="ExternalOutput")

    from contextlib import ExitStack
    with tile.TileContext(nc) as tc, ExitStack() as ctx:
        cpool = ctx.enter_context(tc.tile_pool(name="consts", bufs=1))
        x16p = ctx.enter_context(tc.tile_pool(name="x16", bufs=4))
        xselp = ctx.enter_context(tc.tile_pool(name="xsel4", bufs=2))
        xsqp = ctx.enter_context(tc.tile_pool(name="xsq4", bufs=2))
        sqp = ctx.enter_context(tc.tile_pool(name="sq16", bufs=2))
        zp = ctx.enter_context(tc.tile_pool(name="zout", bufs=3))
        ybp = ctx.enter_context(tc.tile_pool(name="ysb", bufs=2))
        gp = ctx.enter_context(tc.tile_pool(name="gwork", bufs=2))
        smallp = ctx.enter_context(tc.tile_pool(name="small", bufs=4))
        # PSUM: tag "s" = [128,1024] f32 (2 banks) x2 bufs; tag "q" =
        # [128,2048] f32 (4 banks) x1 buf.  8 banks total.
        ps = ctx.enter_context(tc.tile_pool(name="ps", bufs=2, space="PSUM"))

        dma_out = nc.sync.dma_start          # outputs own the sync queue
        dma_in = nc.scalar.dma_start         # inputs on the ACT HWDGE queue

        # ---- constants first on the input queue (they gate the top-k
        # chain), then x tiles ----
        xsumt = cpool.tile([128, 2 * SPC], F32, tag="xsumt")
        dma_in(out=xsumt[:], in_=xsum_d[:, :])
        cb32 = cpool.tile([128, CB32], F32, tag="cb32")
        dma_in(out=cb32[:], in_=cb32_d[:, :])
        cb16 = cpool.tile([128, CB16], F16, tag="cb16")
        dma_in(out=cb16[:], in_=cb16_d[:, :])

        # HAM warmup fodder: a zeroed fp16 tile the PE chews on during the
        # x-load dead time so the clock gate flips to 8/8 before real work.
        warm = cpool.tile([128, 512], F16, tag="warm")
        nc.vector.memset(warm[:], 0.0)
        warm_ps = ps.tile([128, 1024], F32, tag="q", bufs=2, name="warm_ps")

        def ph_warm(n):
            for i in range(n):
                nc.tensor.matmul(warm_ps[:, (i % 2) * 512:(i % 2 + 1) * 512],
                                 lhsT=warm[:, 0:128], rhs=warm[:, 0:512],
                                 start=True, stop=True)

        HPIX = PIX // 2
        X16 = [[None] * 2 for _ in range(SPC)]
        for s in range(SPC):
            for kt in range(2):
                xt = x16p.tile([128, PIX], F16, name="xt")
                r0 = s * C1 + kt * 128
                dma_in(out=xt[:, :HPIX], in_=x_d[r0:r0 + 128, :HPIX])
                dma_in(out=xt[:, HPIX:], in_=x_d[r0:r0 + 128, HPIX:])
                X16[s][kt] = xt

        def c32(col, w):
            return cb32[:, col:col + w]

        wyT32s = [c32(_WY32 + k * 256, 256) for k in range(2)]
        cmat = [c32(_CMAT + k * 256, 256) for k in range(2)]
        tril = [c32(_TRIL + k * 256, 256) for k in range(2)]
        offd = [c32(_OFFD + k * 256, 256) for k in range(2)]
        bcol = [c32(_BCOL + k, 1) for k in range(2)]
        shcol = [c32(_SHCOL + m, 1) for m in range(MT4)]
        iota128 = c32(_IOTA, 128)
        wyT16 = [cb16[:, _WY16 + k * 256: _WY16 + (k + 1) * 256]
                 for k in range(2)]
        wfold16 = [cb16[:, _WFOLD + k * 256: _WFOLD + (k + 1) * 256]
                   for k in range(2)]
        bcol16 = [cb16[:, _BC16 + k: _BC16 + k + 1] for k in range(2)]
        wpair = [cb16[:, _WPAIR + m * 128: _WPAIR + (m + 1) * 128]
                 for m in range(MT4)]

        WSEL4 = [None] * SPC
        SBIAS4 = [None] * SPC
        XSEL4 = [None] * SPC
        XSQ4 = [None] * SPC
        YSB = [[None] * 2 for _ in range(SPC)]
        ZO = [[None] * MT4 for _ in range(SPC)]

        def ph_sel(s):
            # pooled = W'@xbar + b' (exact f32; wyT32s folds the /4096)
            pooled = []
            for mt in range(2):
                pp = ps.tile([128, 1], F32, tag="s")
                for kt in range(2):
                    nc.tensor.matmul(
                        pp[:], lhsT=wyT32s[kt][:, mt * 128:(mt + 1) * 128],
                        rhs=xsumt[:, s * 2 + kt: s * 2 + kt + 1],
                        start=(kt == 0), stop=(kt == 1))
                pb = smallp.tile([128, 1], F32, tag="pooled")
                nc.scalar.activation(pb[:], pp[:], AF.Identity,
                                     bias=bcol[mt], scale=1.0)
                pooled.append(pb)

            lr_ps = ps.tile([1, C1], F32, tag="s")
            for ot in range(2):
                nc.tensor.matmul(lr_ps[:], lhsT=pooled[ot][:], rhs=cmat[ot],
                                 start=(ot == 0), stop=(ot == 1))
            lrow = smallp.tile([1, C1], F32, tag="lrow")
            nc.scalar.copy(lrow[:], lr_ps[:])

            st4 = []
            for qt in range(2):
                lc_ps = ps.tile([128, 1], F32, tag="s")
                for ot in range(2):
                    nc.tensor.matmul(
                        lc_ps[:], lhsT=cmat[ot][:, qt * 128:(qt + 1) * 128],
                        rhs=pooled[ot][:], start=(ot == 0), stop=(ot == 1))
                lcol = smallp.tile([128, 1], F32, tag="lcol")
                nc.scalar.copy(lcol[:], lc_ps[:])

                # exact broadcast of logits row to all partitions
                brow = gp.tile([128, C1], F32, tag="brow")
                nc.gpsimd.partition_broadcast(brow[:], lrow[:])
                # rank[a] = #{b!=a: logits[b] > logits[a]}
                #        + #{b < a: logits[b] == logits[a]}   (jax tie-break)
                # offd kills the diagonal: lrow/lcol come from different
                # matmuls so brow[a,a] vs lcol[a] can misfire by 1 ulp.
                g2 = gp.tile([128, C1], F32)
                nc.vector.scalar_tensor_tensor(
                    g2[:], brow[:], lcol[:], tril[qt],
                    op0=ALU.is_equal, op1=ALU.mult)
                gsum = gp.tile([128, C1], F32)
                nc.vector.scalar_tensor_tensor(
                    gsum[:], brow[:], lcol[:], g2[:],
                    op0=ALU.is_gt, op1=ALU.add)
                gm = gp.tile([128, C1], F32)
                nc.vector.tensor_tensor(gm[:], gsum[:], offd[qt], op=ALU.mult)
                rank = smallp.tile([128, 1], F32, tag="rank")
                nc.vector.tensor_reduce(rank[:], gm[:],
                                        axis=mybir.AxisListType.X, op=ALU.add)
                # S4_T[c, 32g+k] = (rank[c] == k), 4 horizontal copies
                stq = smallp.tile([128, 128], F16, tag="st", bufs=4)
                nc.vector.tensor_scalar(stq[:], iota128, rank[:], None,
                                        op0=ALU.is_equal)
                st4.append(stq)
                if dbg:
                    r0 = s * C1 + qt * 128
                    dma_out(out=dbg_rank[r0:r0 + 128, :], in_=rank[:])

            # selection weights (4 copies): W_sel4T[c, 32g+k]
            wsel4 = []
            for ct in range(2):
                ws_ps = ps.tile([128, 128], F32, tag="s")
                for ot in range(2):
                    nc.tensor.matmul(
                        ws_ps[:], lhsT=wfold16[ot][:, ct * 128:(ct + 1) * 128],
                        rhs=st4[ot][:], start=(ot == 0), stop=(ot == 1))
                wsq = smallp.tile([128, 128], F16, tag="wsel", bufs=4)
                nc.scalar.copy(wsq[:], ws_ps[:])
                wsel4.append(wsq)
            WSEL4[s] = wsel4
            sb_ps = ps.tile([128, 1], F32, tag="s")
            for ot in range(2):
                nc.tensor.matmul(sb_ps[:], lhsT=st4[ot][:], rhs=bcol16[ot],
                                 start=(ot == 0), stop=(ot == 1))
            sbias4 = smallp.tile([128, 1], F32, tag="sbias")
            nc.scalar.copy(sbias4[:], sb_ps[:])
            SBIAS4[s] = sbias4

        def ph_xsel4(s):
            # xsel4 = 4 partition-copies of (W_sel @ x + S b'), via Wsel4.
            xsel4 = xselp.tile([128, PIX], F16, name="xsel4")
            for t in range(4):
                xt = ps.tile([128, 1024], F32, tag="s")
                for kt in range(2):          # same weights consecutive
                    for j in range(2):
                        w = t * 2 + j
                        nc.tensor.matmul(
                            xt[:, j * 512:(j + 1) * 512],
                            lhsT=WSEL4[s][kt][:],
                            rhs=X16[s][kt][:, w * 512:(w + 1) * 512],
                            start=(kt == 0), stop=(kt == 1))
                nc.scalar.activation(xsel4[:, t * 1024:(t + 1) * 1024],
                                     xt[:], AF.Identity,
                                     bias=SBIAS4[s][:], scale=1.0)
            XSEL4[s] = xsel4
            if dbg:
                dma_out(out=dbg_xsel4[s * 128:(s + 1) * 128, :],
                        in_=xsel4[:])

        def ph_xsq4(s, eng):
            # xsq4 = xsel4^2 (fp16).  Only partitions 64:128 feed the Q
            # matmuls but computing all 128 costs the same (free-dim bound).
            xsq4 = xsqp.tile([128, PIX], F16, name="xsq4")
            if eng == "G":
                nc.gpsimd.tensor_tensor(xsq4[:], XSEL4[s][:], XSEL4[s][:],
                                        op=ALU.mult)
            else:
                nc.vector.tensor_tensor(xsq4[:], XSEL4[s][:], XSEL4[s][:],
                                        op=ALU.mult)
            XSQ4[s] = xsq4

        def ph_pair_unit(s, m, u):
            # 2 windows of 512 via one 4-way row-tiled matmul quad:
            #   S(w0)@rows0-31, S(w1)@rows32-63, Q(w0)@rows64-95,
            #   Q(w1)@rows96-127  (concurrent on the PE sub-arrays)
            # z = (Qneg + sh) + Square(S)
            p = min(128, CSE - m * 128)
            wp = wpair[m]
            xsel4 = XSEL4[s]
            xsq4 = XSQ4[s]
            if ZO[s][m] is None:
                ZO[s][m] = zp.tile([128, PIX], F16, name=f"zo{s}{m}",
                                   tag="zo")
            zo = ZO[s][m]
            sqT = sqp.tile([128, 1024], F16, tag="sq", bufs=3)
            sU = ps.tile([128, 1024], F32, tag="s")
            qU = ps.tile([128, 1024], F32, tag="q")
            w0 = u * 2
            for j in range(2):
                g = j                                # row groups 0,1 -> S
                nc.tensor.matmul(
                    sU[0:p, j * 512:(j + 1) * 512],
                    lhsT=wp[32 * g:32 * (g + 1), 0:p],
                    rhs=xsel4[32 * g:32 * (g + 1),
                              (w0 + j) * 512:(w0 + j + 1) * 512],
                    start=True, stop=True,
                    tile_position=(32 * g, 0))
            for j in range(2):
                g = 2 + j                            # row groups 2,3 -> Qneg
                nc.tensor.matmul(
                    qU[0:p, j * 512:(j + 1) * 512],
                    lhsT=wp[32 * g:32 * (g + 1), 0:p],
                    rhs=xsq4[32 * g:32 * (g + 1),
                             (w0 + j) * 512:(w0 + j + 1) * 512],
                    start=True, stop=True,
                    tile_position=(32 * g, 0))
            nc.scalar.activation(sqT[0:p, :], sU[0:p, :], AF.Square)
            nc.vector.scalar_tensor_tensor(
                zo[0:p, w0 * 512:(w0 + 2) * 512],
                qU[0:p, :], shcol[m][0:p, :], sqT[0:p, :],
                op0=ALU.add, op1=ALU.add)
            if u == 3:
                r0 = s * COUT + C1 + m * 128
                dma_out(out=out_d[r0:r0 + p, :], in_=zo[0:p, :])

        def ph_pair_half(s, m, half):
            ph_pair_unit(s, m, half * 2)
            ph_pair_unit(s, m, half * 2 + 1)

        def ph_y_unit(s, mt, t, eng):
            # one [128,1024] y unit: y = W'x + b' -> fp16 staging
            if YSB[s][mt] is None:
                YSB[s][mt] = ybp.tile([128, PIX], F16, name=f"ysb{s}{mt}",
                                      tag="ysb")
            ysb = YSB[s][mt]
            yt = ps.tile([128, 1024], F32, tag="s")
            for kt in range(2):              # same weights consecutive
                for j in range(2):
                    w = t * 2 + j
                    nc.tensor.matmul(
                        yt[:, j * 512:(j + 1) * 512],
                        lhsT=wyT16[kt][:, mt * 128:(mt + 1) * 128],
                        rhs=X16[s][kt][:, w * 512:(w + 1) * 512],
                        start=(kt == 0), stop=(kt == 1))
            dst = ysb[:, t * 1024:(t + 1) * 1024]
            if eng == "A":
                nc.scalar.activation(dst, yt[:], AF.Identity,
                                     bias=bcol[mt], scale=1.0)
            else:
                nc.vector.tensor_scalar(dst, yt[:], bcol[mt], None,
                                        op0=ALU.add)
            if t == 3:
                r0 = s * COUT + mt * 128
                dma_out(out=out_d[r0:r0 + 128, :], in_=ysb[:])

        # ---- emission schedule (per-engine FIFO order) ----
        ph_warm(14)
        ph_sel(0)
        ph_warm(6)
        ph_xsel4(0)
        ph_xsq4(0, "V")
        ph_sel(1)
        ph_warm(6)
        ph_y_unit(0, 0, 0, "A")
        ph_y_unit(0, 0, 1, "V")
        ph_pair_half(0, 0, 0)
        ph_pair_half(0, 0, 1)
        ph_xsel4(1)
        ph_xsq4(1, "G")
        ph_y_unit(0, 0, 2, "A")
        ph_y_unit(0, 0, 3, "V")
        ph_pair_half(0, 1, 0)
        ph_pair_half(0, 1, 1)
        ph_y_unit(0, 1, 0, "A")
        ph_y_unit(0, 1, 1, "V")
        ph_pair_half(0, 2, 0)
        ph_pair_half(0, 2, 1)
        ph_y_unit(0, 1, 2, "A")
        ph_y_unit(0, 1, 3, "V")
        ph_pair_half(0, 3, 0)
        ph_pair_half(0, 3, 1)
        ph_y_unit(1, 0, 0, "A")
        ph_y_unit(1, 0, 1, "V")
        ph_pair_half(1, 0, 0)
        ph_pair_half(1, 0, 1)
        ph_y_unit(1, 0, 2, "A")
        ph_y_unit(1, 0, 3, "V")
        ph_pair_half(1, 1, 0)
        ph_pair_half(1, 1, 1)
        ph_y_unit(1, 1, 0, "A")
        ph_y_unit(1, 1, 1, "V")
        ph_pair_half(1, 2, 0)
        ph_pair_half(1, 2, 1)
        ph_y_unit(1, 1, 2, "A")
        ph_y_unit(1, 1, 3, "V")
        ph_pair_half(1, 3, 0)
        ph_pair_half(1, 3, 1)


_CACHE = {}


def _get_nc(dbg: bool = False):
    key = f"nc{int(dbg)}"
    if key not in _CACHE:
        nc = bacc.Bacc("TRN2", target_bir_lowering=False, debug=False,
                       num_devices=NCORES)
        _build(nc, dbg=dbg)
        nc.compile()
        _CACHE[key] = nc
    return _CACHE[key]


def _host_params(w_fc, b_fc, g_x, b_x, m_x, v_x, eca_w, g_s, b_s, m_s, v_s):
    sx = (g_x / np.sqrt(v_x + EPS)).astype(np.float32)            # [256]
    Wp = (sx[:, None] * w_fc).astype(np.float32)                  # [o, c]
    bp = (sx * b_fc + b_x - m_x * sx).astype(np.float32)          # [256]

    cmat = np.zeros((C1, C1), np.float32)                         # [o, q]
    for k in range(5):
        d = k - 2                                                 # o - q
        for q in range(C1):
            o = q + d
            if 0 <= o < C1:
                cmat[o, q] = eca_w[k]

    tril = (np.arange(C1)[None, :] < np.arange(C1)[:, None]).astype(np.float32)
    offd = (1.0 - np.eye(C1, dtype=np.float32))

    hi, hj = np.triu_indices(CS, k=1)
    ss = (g_s / np.sqrt(v_s + EPS)).astype(np.float32)
    sh = (b_s - m_s * ss).astype(np.float32)
    # squares-trick pair weights:
    #   S = w16*(xi + xj), w16 = fp16(sqrt(ss/2))
    #   Qneg = -q16*(xi^2 + xj^2), q16 = fp16(w16^2)  (so S^2+Qneg ~= ss*xi*xj)
    w16 = np.sqrt(ss / 2.0).astype(np.float16)
    q16 = (w16.astype(np.float32) ** 2).astype(np.float16)
    wpair = np.zeros((MT4, 128, 128), np.float16)
    for m in range(MT4):
        p = min(128, CSE - m * 128)
        cols = np.arange(p)
        pq = m * 128 + cols
        blkS = np.zeros((32, 128), np.float32)
        blkS[hi[pq], cols] = w16[pq].astype(np.float32)
        blkS[hj[pq], cols] = w16[pq].astype(np.float32)
        blkQ = np.zeros((32, 128), np.float32)
        blkQ[hi[pq], cols] = -q16[pq].astype(np.float32)
        blkQ[hj[pq], cols] = -q16[pq].astype(np.float32)
        wpair[m][0:32] = blkS
        wpair[m][32:64] = blkS
        wpair[m][64:96] = blkQ
        wpair[m][96:128] = blkQ

    iota128 = np.tile(np.tile(np.arange(CS, dtype=np.float32), 4), (128, 1))

    return {
        "wyT16": Wp.T.astype(np.float16).copy(),
        "wyT32s": (Wp.T / PIX).astype(np.float32).copy(),
        "wfold16": Wp.astype(np.float16).copy(),
        "bcol": bp.reshape(C1, 1).copy(),
        "bcol16": bp.astype(np.float16).reshape(C1, 1).copy(),
        "cmat": cmat,
        "tril": tril,
        "offd": offd,
        "iota128": iota128,
        "wpair": wpair,
        "shcol": sh.reshape(CSE, 1).copy(),
    }


def _semantic_params(inputs):
    return _host_params(
        np.asarray(inputs["w_fc"], np.float32),
        np.asarray(inputs["b_fc"], np.float32),
        np.asarray(inputs["bn_x_gamma"], np.float32),
        np.asarray(inputs["bn_x_beta"], np.float32),
        np.asarray(inputs["bn_x_mean"], np.float32),
        np.asarray(inputs["bn_x_var"], np.float32),
        np.asarray(inputs["eca_w"], np.float32),
        np.asarray(inputs["bn_s_gamma"], np.float32),
        np.asarray(inputs["bn_s_beta"], np.float32),
        np.asarray(inputs["bn_s_mean"], np.float32),
        np.asarray(inputs["bn_s_var"], np.float32),
    )


def _pack_blobs(P):
    """Pack semantic params into the const blobs matching _build's layout."""
    cb32 = np.zeros((128, CB32), np.float32)
    for k in range(2):
        cb32[:, _WY32 + k * 256: _WY32 + (k + 1) * 256] = \
            P["wyT32s"][k * 128:(k + 1) * 128]
        cb32[:, _CMAT + k * 256: _CMAT + (k + 1) * 256] = \
            P["cmat"][k * 128:(k + 1) * 128]
        cb32[:, _TRIL + k * 256: _TRIL + (k + 1) * 256] = \
            P["tril"][k * 128:(k + 1) * 128]
        cb32[:, _OFFD + k * 256: _OFFD + (k + 1) * 256] = \
            P["offd"][k * 128:(k + 1) * 128]
        cb32[:, _BCOL + k] = P["bcol"][k * 128:(k + 1) * 128, 0]
    for m in range(MT4):
        p = min(128, CSE - m * 128)
        cb32[:p, _SHCOL + m] = P["shcol"][m * 128: m * 128 + p, 0]
    cb32[:, _IOTA:_IOTA + 128] = P["iota128"]

    cb16 = np.zeros((128, CB16), np.float16)
    for k in range(2):
        cb16[:, _WY16 + k * 256: _WY16 + (k + 1) * 256] = \
            P["wyT16"][k * 128:(k + 1) * 128]
        cb16[:, _WFOLD + k * 256: _WFOLD + (k + 1) * 256] = \
            P["wfold16"][k * 128:(k + 1) * 128]
        cb16[:, _BC16 + k] = P["bcol16"][k * 128:(k + 1) * 128, 0]
    for m in range(MT4):
        cb16[:, _WPAIR + m * 128: _WPAIR + (m + 1) * 128] = P["wpair"][m]

    return {"cb32": cb32, "cb16": np.ascontiguousarray(cb16)}


def _in_maps(inputs):
    x = np.ascontiguousarray(np.asarray(inputs["x"], np.float32))
    blobs = _pack_blobs(_semantic_params(inputs))
    maps = []
    for c in range(NCORES):
        shard = x[c * SPC:(c + 1) * SPC].reshape(SPC * C1, PIX)
        # exact f32 per-channel pixel sums (feeds the pooled/top-k path);
        # fp16 x feeds the matmuls
        xsum = shard.sum(axis=1, dtype=np.float32)          # [512]
        xsumv = np.zeros((128, 2 * SPC), np.float32)
        for s in range(SPC):
            for kt in range(2):
                xsumv[:, s * 2 + kt] = xsum[s * C1 + kt * 128:
                                            s * C1 + (kt + 1) * 128]
        maps.append({"x16v": shard.astype(np.float16),
                     "xsumv": xsumv, **blobs})
    return maps


def _ensure_ntff_hook():
    """The agent image lacks antenv.axon_hooks; synthesize it so
    run_bass_kernel_spmd(trace=True) can reach the NTFF profiler in
    libaxon_pjrt.so. Safe no-op if anything is missing."""
    try:
        import antenv.axon_hooks  # noqa: F401
        return
    except ImportError:
        pass
    try:
        import types
        import antenv
        from trn_agent_boot.trn_boot import _ntff_profile_via_ctypes
        hook = _ntff_profile_via_ctypes("/opt/axon/libaxon_pjrt.so")
        mod = types.ModuleType("antenv.axon_hooks")
        mod._hook = hook
        mod.get_axon_ntff_profile_hook = lambda: mod._hook
        mod.set_axon_ntff_profile_hook = lambda h: setattr(mod, "_hook", h)
        sys.modules["antenv.axon_hooks"] = mod
        antenv.axon_hooks = mod
    except Exception as e:  # pragma: no cover
        print(f"ntff hook shim failed: {e}", file=sys.stderr)


def run(inputs, trace=False, dbg=False):
    if trace:
        _ensure_ntff_hook()
    nc = _get_nc(dbg=dbg)
    maps = _in_maps(inputs)
    res = run_bass_kernel_spmd(nc, maps, core_ids=list(range(NCORES)),
                               trace=trace)
    outs = [np.asarray(res.results[c]["out"]).astype(np.float32)
            .reshape(SPC, COUT, H, W) for c in range(NCORES)]
    return np.concatenate(outs, axis=0), res


def kernel(**inputs) -> np.ndarray:
    out, _ = run(inputs, trace=False)
    return out
